# revision 4
# baseline (speedup 1.0000x reference)
"""Multi-head causal self-attention for TRN2, 8 NeuronCores.

Sharding: core i handles (batch b = i//2, head-group g = i%2); each head-group
is 8 of the 16 heads.  Per core everything is computed in "transposed" space so
no on-device transposes are needed.

v2 vs baseline:
  * QK attention matmuls are row-tiled: the two heads of a pair run as
    concurrent K=64 matmuls on PE row-groups (0,0)/(64,0) writing the two
    halves (separate PSUM banks) of one [128, 1024] score tile, so both
    heads' S^T cost one 512-col stream instead of two.
  * Single software-pipelined loop: phase-1 QKV projection work for block
    j+1 and the output projection for block j-1 are emitted as fine-grained
    "filler" matmuls inside the ACT(exp)-paced attention chunk loop, with
    FIFO gating so attention never waits on un-emitted producers.
  * No ACT in phase 1: QKV biases (when nonzero) are rank-1 matmuls; PSUM
    drains are DVE copies.  ACT does only the 160 softmax exp calls.
  * x^T f32->bf16 casts run on the otherwise-idle GPSIMD engine.
  * softmax normalization: reciprocal on DVE, [1,q]->[64,q] broadcast via a
    K=1 matmul into the shared PSUM pool, deferred past the next pair's
    first chunk so the PE never stalls on the DVE reciprocal chain.
"""

import numpy as np
import ml_dtypes
from collections import deque
from contextlib import ExitStack

import concourse.bass as bass
import concourse.mybir as mybir
import concourse.tile as tile
from concourse import bacc
from concourse.bass_utils import run_bass_kernel_spmd

B, T, D, H = 4, 2048, 1024, 16
DK = 64            # head dim
HL = 8             # heads per core
DL = HL * DK       # 512 local head dims per core
N_CORES = 8

F32 = mybir.dt.float32
F32R = mybir.dt.float32r
BF16 = mybir.dt.bfloat16
EXP = mybir.ActivationFunctionType.Exp

TQ = 512           # tq block size
TKC = 128          # tk chunk size
NQB = T // TQ      # 4
NKC = T // TKC     # 16
NDCH = D // 128    # 8 contraction chunks over D
VSW = HL * 65 + 64  # staged-V width: 8*[V_h|1] + ones tail pad for M=128 lhsT

# rough per-instruction engine-busy estimates (ns) for the static scheduler
EST_QK = 250       # two concurrent row-tiled K=64 matmuls, N=512
EST_PV = 430       # two K=128 matmuls, N=512
EST_EXP = 1150     # ACT exp on [128, 1024]
EST_FILL = 215     # one N=512 matmul
RESERVE = 700

_CACHE = {}


def _build(causal: bool, qkv_bias: bool):
    nc = bacc.Bacc("TRN2", target_bir_lowering=False, debug=False,
                   num_devices=N_CORES)
    xT_d = nc.dram_tensor("xT", [D, T], F32, kind="ExternalInput").ap()
    wqk_d = nc.dram_tensor("wqk", [D, 2 * DL], F32, kind="ExternalInput").ap()
    wv_d = nc.dram_tensor("wv", [D, DL], F32, kind="ExternalInput").ap()
    wp_d = nc.dram_tensor("wproj", [DL, D], F32, kind="ExternalInput").ap()
    bqk_d = nc.dram_tensor("bqk", [1, 2 * DL], F32, kind="ExternalInput").ap()
    bv_d = nc.dram_tensor("bv", [1, DL], F32, kind="ExternalInput").ap()
    masks_d = nc.dram_tensor("masks", [TKC, 4 * TQ], BF16,
                             kind="ExternalInput").ap()
    out_d = nc.dram_tensor("out", [T, D], F32, kind="ExternalOutput").ap()

    with tile.TileContext(nc) as tc, ExitStack() as top:
        persist = top.enter_context(tc.tile_pool(name="persist", bufs=1))
        wstage = top.enter_context(tc.tile_pool(name="wstage", bufs=1))
        xstage = top.enter_context(tc.tile_pool(name="xstage", bufs=1))
        xrpool = top.enter_context(
            tc.tile_pool(name="xrpool", bufs=2 if causal else 4))
        ps_s = top.enter_context(tc.tile_pool(name="ps_s", bufs=2, space="PSUM"))
        ps_o = top.enter_context(tc.tile_pool(name="ps_o", bufs=2, space="PSUM"))
        ps_sh = top.enter_context(tc.tile_pool(name="ps_sh", bufs=2, space="PSUM"))
        ppool = top.enter_context(tc.tile_pool(name="ppool", bufs=6))
        npool = top.enter_context(tc.tile_pool(name="npool", bufs=2))
        opool = top.enter_context(tc.tile_pool(name="opool", bufs=3))

        # ---------------- persistent tiles ----------------
        q2 = [persist.tile([128, T], BF16, tag=f"q2{i}", name=f"q2{i}")
              for i in range(4)]       # head-pair packed Q^T
        kT = [persist.tile([128, T], BF16, tag=f"kT{i}", name=f"kT{i}")
              for i in range(4)]       # head-pair packed K^T
        vs = [persist.tile([128, VSW], BF16, tag=f"vs{t}", name=f"vs{t}")
              for t in range(NKC)]     # staged V: [V_h|1]*8 + ones tail
        yT = [persist.tile([128, T], BF16, tag=f"yT{i}", name=f"yT{i}")
              for i in range(4)]
        wqk_r = [persist.tile([128, 2 * DL], BF16, tag=f"wqk{d}", name=f"wqk{d}")
                 for d in range(NDCH)]
        wv_r = [persist.tile([128, DL], BF16, tag=f"wv{d}", name=f"wv{d}")
                for d in range(NDCH)]
        wp_r = [persist.tile([128, D], BF16, tag=f"wp{k}", name=f"wp{k}")
                for k in range(4)]
        ones_r = persist.tile([1, 128], F32R, tag="ones_r", name="ones_r")
        maskb = None
        if causal:
            maskb = persist.tile([TKC, 4 * TQ], BF16, tag="maskb", name="maskb")
            nc.gpsimd.dma_start(maskb[:], masks_d)

        # ---------------- preamble ----------------
        initp = top.enter_context(tc.tile_pool(name="initp", bufs=1))
        ones_f = initp.tile([1, 512], F32, tag="ones_f", name="ones_f")
        nc.vector.memset(ones_f[:], 1.0)
        nc.vector.tensor_copy(ones_r[:], ones_f[:, 0:128])
        bqk_r = bv_r = ones512_r = None
        if qkv_bias:
            ones512_r = initp.tile([1, 512], F32R, tag="ones512_r",
                                   name="ones512_r")
            nc.vector.tensor_copy(ones512_r[:], ones_f[:])
            bqk_f = initp.tile([1, 2 * DL], F32, tag="bqk_f", name="bqk_f")
            nc.gpsimd.dma_start(bqk_f[:], bqk_d)
            bqk_r = initp.tile([1, 2 * DL], F32R, tag="bqk_r", name="bqk_r")
            nc.vector.tensor_copy(bqk_r[:], bqk_f[:])
            bv_f = initp.tile([1, DL], F32, tag="bv_f", name="bv_f")
            nc.gpsimd.dma_start(bv_f[:], bv_d)
            bv_r = initp.tile([1, DL], F32R, tag="bv_r", name="bv_r")
            nc.vector.tensor_copy(bv_r[:], bv_f[:])

        # weights: DMA stage f32 -> DVE cast to bf16 resident copies
        for d in range(NDCH):
            st = wstage.tile([128, 2 * DL], F32, tag="wqks", name=f"wqks{d}")
            nc.gpsimd.dma_start(st[:], wqk_d[d * 128:(d + 1) * 128, :])
            nc.vector.tensor_copy(wqk_r[d][:], st[:])
            stv = wstage.tile([128, DL], F32, tag="wvs", name=f"wvs{d}")
            nc.gpsimd.dma_start(stv[:], wv_d[d * 128:(d + 1) * 128, :])
            nc.vector.tensor_copy(wv_r[d][:], stv[:])
        for k in range(4):
            st = wstage.tile([128, D], F32, tag="wps", name=f"wps{k}")
            nc.gpsimd.dma_start(st[:], wp_d[k * 128:(k + 1) * 128, :])
            nc.vector.tensor_copy(wp_r[k][:], st[:])

        # staged-V tiles start as all-ones; the V copies overwrite the V
        # columns and leave the |1 columns and the tail as ones.
        for t in range(NKC):
            nc.gpsimd.memset(vs[t][:], 1.0)

        # ---------------- x loads (DMA + gpsimd cast) ----------------
        xr_cache = {}

        def xload(j):
            jsl = slice(j * TQ, (j + 1) * TQ)
            xr_j = []
            for d in range(NDCH):
                st = xstage.tile([128, TQ], F32, tag=f"xs{d}", name=f"xs{j}_{d}")
                nc.sync.dma_start(st[:], xT_d[d * 128:(d + 1) * 128, jsl])
                xr_t = xrpool.tile([128, TQ], BF16, tag=f"xr{d}",
                                   name=f"xr{j}_{d}")
                nc.gpsimd.tensor_copy(xr_t[:], st[:])
                xr_j.append(xr_t)
            xr_cache[j] = xr_j

        # ---------------- filler machinery ----------------
        filler = deque()   # items: (label_or_None, fn, est_pe_ns)
        done = set()
        est = {"pe": 0.0, "act": 0.0}

        def pop_one():
            label, fn, cost = filler.popleft()
            fn()
            if label is not None:
                done.add(label)
            est["pe"] += cost

        def need(label):
            while label not in done:
                assert filler, f"gate {label} not in filler"
                pop_one()

        def budget_pops():
            while filler and est["pe"] + RESERVE < est["act"]:
                pop_one()

        def ph1_steps(j):
            """Phase-1 QKV projection for query block j as filler items."""
            jsl = slice(j * TQ, (j + 1) * TQ)
            xr_j = xr_cache[j]
            items = []

            def m_group(m):
                cell = {}

                def mk(d):
                    def fn():
                        if d == 0:
                            cell["ps"] = ps_sh.tile(
                                [128, TQ], F32, tag="sh", name=f"psqk{j}_{m}")
                        ps = cell["ps"]
                        last = (d == NDCH - 1) and not qkv_bias
                        nc.tensor.matmul(
                            ps[:], wqk_r[d][:, m * 128:(m + 1) * 128],
                            xr_j[d][:], start=(d == 0), stop=last)
                        if d == NDCH - 1:
                            if qkv_bias:
                                nc.tensor.matmul(
                                    ps[:], bqk_r[0:1, m * 128:(m + 1) * 128],
                                    ones512_r[:], start=False, stop=True)
                            dst = q2[m] if m < 4 else kT[m - 4]
                            nc.vector.tensor_copy(dst[:, jsl], ps[:])
                    return fn

                return ([(None, mk(d), EST_FILL) for d in range(NDCH - 1)]
                        + [(("m", j, m), mk(NDCH - 1), EST_FILL + 60)])

            def v_group(tt):
                c = tt % 4
                cell = {}

                def mk(d):
                    def fn():
                        if d == 0:
                            cell["ps"] = ps_sh.tile(
                                [128, DL], F32, tag="sh", name=f"psv{tt}")
                        ps = cell["ps"]
                        last = (d == NDCH - 1) and not qkv_bias
                        nc.tensor.matmul(
                            ps[:], xr_j[d][:, c * 128:(c + 1) * 128],
                            wv_r[d][:], start=(d == 0), stop=last)
                        if d == NDCH - 1:
                            if qkv_bias:
                                nc.tensor.matmul(
                                    ps[:], ones_r[:, 0:128], bv_r[:],
                                    start=False, stop=True)
                            src = ps.rearrange("p (h x) -> p h x", h=HL)
                            dst = vs[tt][:, 0:HL * 65].rearrange(
                                "p (h x) -> p h x", x=65)[:, :, 0:64]
                            nc.vector.tensor_copy(dst, src)
                    return fn

                return ([(None, mk(d), EST_FILL) for d in range(NDCH - 1)]
                        + [(("v", j, tt % 4), mk(NDCH - 1), EST_FILL + 60)])

            # order: what attention needs first -- pair-0 Q/K, then V, then
            # the remaining pairs' Q/K.
            items += m_group(0) + m_group(4)
            for tt in range(4 * j, 4 * j + 4):
                items += v_group(tt)
            for i in range(1, 4):
                items += m_group(i) + m_group(4 + i)
            return items

        def proj_steps(j):
            items = []
            for t in range(4 * j, 4 * j + 4):
                for nb in range(2):
                    def fn(t=t, nb=nb):
                        nsl = slice(nb * 512, (nb + 1) * 512)
                        ps3 = ps_sh.tile([128, TQ], F32, tag="sh",
                                         name=f"ps3_{t}_{nb}")
                        for k in range(4):
                            nc.tensor.matmul(
                                ps3[:], yT[k][:, t * 128:(t + 1) * 128],
                                wp_r[k][:, nsl], start=(k == 0), stop=(k == 3))
                        ot = opool.tile([128, TQ], F32, tag="ot",
                                        name=f"ot{t}_{nb}")
                        nc.vector.tensor_copy(ot[:], ps3[:])
                        nc.sync.dma_start(out_d[t * 128:(t + 1) * 128, nsl],
                                          ot[:])
                    items.append((None, fn, 4 * EST_FILL))
            return items

        norm_q = deque()   # deferred [pb matmul + yT mul] closures

        def drain_norm():
            while norm_q:
                norm_q.popleft()()
                est["pe"] += 2 * EST_FILL

        # ---------------- main pipelined loop ----------------
        xload(0)
        if causal:
            xload(1)
            filler.extend(ph1_steps(0))
        else:
            for j in range(1, NQB):
                xload(j)
            for j in range(NQB):
                filler.extend(ph1_steps(j))

        for j in range(NQB):
            jsl = slice(j * TQ, (j + 1) * TQ)
            cs = list(range(4 * (j + 1))) if causal else list(range(NKC))
            if causal and j + 2 < NQB:
                xload(j + 2)
            if causal and j + 1 < NQB:
                filler.extend(ph1_steps(j + 1))

            for i in range(4):          # head pair (2i, 2i+1)
                hA, hB = 2 * i, 2 * i + 1
                need(("m", j, i))
                poA = ps_o.tile([128, TQ], F32, tag="po", name=f"poA{j}_{i}")
                poB = ps_o.tile([128, TQ], F32, tag="po", name=f"poB{j}_{i}")

                pend = None   # pipeline: PV(c) emitted after QK(c+1)
                for ci, c in enumerate(cs):
                    need(("m", c // 4, 4 + i))
                    csl = slice(c * TKC, (c + 1) * TKC)
                    ss = ps_s.tile([TKC, 2 * TQ], F32, tag="ss",
                                   name=f"ss{j}_{i}_{c}")
                    nc.tensor.matmul(ss[:, 0:TQ], kT[i][0:64, csl],
                                     q2[i][0:64, jsl], start=True, stop=True)
                    nc.tensor.matmul(ss[:, TQ:2 * TQ], kT[i][64:128, csl],
                                     q2[i][64:128, jsl], start=True, stop=True)
                    est["pe"] += EST_QK
                    pt = ppool.tile([TKC, 2 * TQ], BF16, tag="pt",
                                    name=f"pt{j}_{i}_{c}")
                    nc.scalar.activation(pt[:], ss[:], EXP, scale=0.125)
                    est["act"] += EST_EXP
                    if causal and c >= 4 * j:
                        s = c - 4 * j
                        msl = slice(s * TQ, (s + 1) * TQ)
                        nc.vector.tensor_mul(pt[:, 0:TQ], pt[:, 0:TQ],
                                             maskb[:, msl])
                        nc.vector.tensor_mul(pt[:, TQ:2 * TQ],
                                             pt[:, TQ:2 * TQ], maskb[:, msl])
                    if ci == 0:
                        drain_norm()   # previous pair's deferred norm
                    if pend is not None:
                        pc, ppt = pend
                        need(("v", pc // 4, pc % 4))
                        st = (ci == 1)
                        nc.tensor.matmul(
                            poA[:], vs[pc][:, hA * 65:hA * 65 + 128],
                            ppt[:, 0:TQ], start=st, stop=False)
                        nc.tensor.matmul(
                            poB[:], vs[pc][:, hB * 65:hB * 65 + 128],
                            ppt[:, TQ:2 * TQ], start=st, stop=False)
                        est["pe"] += EST_PV
                    pend = (c, pt)
                    budget_pops()
                pc, ppt = pend
                need(("v", pc // 4, pc % 4))
                one = (len(cs) == 1)
                nc.tensor.matmul(poA[:], vs[pc][:, hA * 65:hA * 65 + 128],
                                 ppt[:, 0:TQ], start=one, stop=True)
                nc.tensor.matmul(poB[:], vs[pc][:, hB * 65:hB * 65 + 128],
                                 ppt[:, TQ:2 * TQ], start=one, stop=True)
                est["pe"] += EST_PV

                # immediate DVE part of softmax normalization
                rr, osb = [], []
                for h, po in ((hA, poA), (hB, poB)):
                    sums = npool.tile([1, TQ], F32, tag="sums",
                                      name=f"sm{j}_{h}")
                    nc.vector.tensor_copy(sums[:], po[64:65, :])
                    o_sb = npool.tile([64, TQ], BF16, tag="o_sb",
                                      name=f"ob{j}_{h}")
                    nc.vector.tensor_copy(o_sb[:], po[0:64, :])
                    recip = npool.tile([1, TQ], F32, tag="recip",
                                       name=f"rc{j}_{h}")
                    scr = npool.tile([1, TQ], F32, tag="scr", name=f"sc{j}_{h}")
                    recip_r = npool.tile([1, TQ], F32R, tag="recip_r",
                                         name=f"rr{j}_{h}")
                    nc.vector.reciprocal_approx_accurate(
                        out=recip[:], in_=sums[:], scratch=scr[:])
                    nc.vector.tensor_copy(recip_r[:], recip[:])
                    rr.append(recip_r)
                    osb.append(o_sb)

                def norm_fn(i=i, jsl=jsl, osb=osb, rr=rr, j=j, hA=hA):
                    for hp in (0, 1):
                        pb = ps_sh.tile([64, TQ], F32, tag="sh",
                                        name=f"pb{j}_{hA + hp}")
                        nc.tensor.matmul(pb[:], ones_r[:, 0:64], rr[hp][:],
                                         start=True, stop=True)
                        nc.vector.tensor_mul(
                            yT[i][hp * 64:(hp + 1) * 64, jsl],
                            osb[hp][:], pb[:])
                norm_q.append(norm_fn)

            drain_norm()               # before proj(j) can be emitted
            filler.extend(proj_steps(j))

        while filler:                  # flush remaining projection work
            pop_one()

    nc.compile()
    return nc


def _get_nc(causal: bool, qkv_bias: bool = False):
    key = (causal, qkv_bias)
    if key not in _CACHE:
        _CACHE[key] = _build(causal, qkv_bias)
    return _CACHE[key]


def _host_masks() -> np.ndarray:
    i = np.arange(TKC)[:, None]
    jj = np.arange(TQ)[None, :]
    blocks = [(jj >= i + s * TKC).astype(np.float32) for s in range(4)]
    return np.ascontiguousarray(
        np.concatenate(blocks, axis=1).astype(ml_dtypes.bfloat16))


def _make_in_maps(x, W_qkv, b_qkv, W_proj):
    masks_np = _host_masks()
    in_maps = []
    for core in range(N_CORES):
        b, g = core // 2, core % 2
        qc = slice(g * DL, (g + 1) * DL)
        kc = slice(D + g * DL, D + (g + 1) * DL)
        vc = slice(2 * D + g * DL, 2 * D + (g + 1) * DL)
        in_maps.append({
            "xT": np.ascontiguousarray(x[b].T),
            "wqk": np.ascontiguousarray(
                np.concatenate([W_qkv[:, qc], W_qkv[:, kc]], axis=1)),
            "wv": np.ascontiguousarray(W_qkv[:, vc]),
            "bqk": np.ascontiguousarray(
                np.concatenate([b_qkv[qc], b_qkv[kc]]).reshape(1, 2 * DL)),
            "bv": np.ascontiguousarray(b_qkv[vc].reshape(1, DL)),
            "wproj": np.ascontiguousarray(W_proj[g * DL:(g + 1) * DL, :]),
            "masks": masks_np,
        })
    return in_maps


def kernel(x, mask, W_qkv, b_qkv, W_proj, b_proj):
    x = np.asarray(x, dtype=np.float32)
    mask2d = np.asarray(mask, dtype=np.int32).reshape(T, T)
    W_qkv = np.asarray(W_qkv, dtype=np.float32)
    b_qkv = np.asarray(b_qkv, dtype=np.float32)
    W_proj = np.asarray(W_proj, dtype=np.float32)
    b_proj = np.asarray(b_proj, dtype=np.float32)

    if np.array_equal(mask2d, np.tril(np.ones((T, T), dtype=np.int32))):
        causal = True
    elif np.all(mask2d == 1):
        causal = False
    else:
        raise NotImplementedError("only causal (tril) or all-ones masks")

    qkv_bias = bool(np.any(b_qkv != 0.0))
    nc = _get_nc(causal, qkv_bias)
    in_maps = _make_in_maps(x, W_qkv, b_qkv, W_proj)
    res = run_bass_kernel_spmd(nc, in_maps, core_ids=list(range(N_CORES)))
    out = np.empty((B, T, D), dtype=np.float32)
    for b in range(B):
        out[b] = (res.results[2 * b]["out"] + res.results[2 * b + 1]["out"]
                  + b_proj[None, :])
    return out


# revision 5
# speedup vs baseline: 1.2181x; 1.2181x over previous
"""Multi-head causal self-attention for TRN2, 8 NeuronCores.

Sharding: core i handles (batch b = i//2, head-group g = i%2); each head-group
is 8 of the 16 heads.  Per core everything is computed in "transposed" space so
no on-device transposes are needed.

v2 vs baseline:
  * QK attention matmuls are row-tiled: the two heads of a pair run as
    concurrent K=64 matmuls on PE row-groups (0,0)/(64,0) writing the two
    halves (separate PSUM banks) of one [128, 1024] score tile, so both
    heads' S^T cost one 512-col stream instead of two.
  * Single software-pipelined loop: phase-1 QKV projection work for block
    j+1 and the output projection for block j-1 are emitted as fine-grained
    "filler" matmuls inside the ACT(exp)-paced attention chunk loop, with
    FIFO gating so attention never waits on un-emitted producers.
  * No ACT in phase 1: QKV biases (when nonzero) are rank-1 matmuls; PSUM
    drains are DVE copies.  ACT does only the 160 softmax exp calls.
  * x^T f32->bf16 casts run on the otherwise-idle GPSIMD engine.
  * softmax normalization: reciprocal on DVE, [1,q]->[64,q] broadcast via a
    K=1 matmul into the shared PSUM pool, deferred past the next pair's
    first chunk so the PE never stalls on the DVE reciprocal chain.
"""

import numpy as np
import ml_dtypes
from collections import deque
from contextlib import ExitStack

import concourse.bass as bass
import concourse.mybir as mybir
import concourse.tile as tile
from concourse import bacc
from concourse.bass_utils import run_bass_kernel_spmd

B, T, D, H = 4, 2048, 1024, 16
DK = 64            # head dim
HL = 8             # heads per core
DL = HL * DK       # 512 local head dims per core
N_CORES = 8

F32 = mybir.dt.float32
F32R = mybir.dt.float32r
BF16 = mybir.dt.bfloat16
EXP = mybir.ActivationFunctionType.Exp

TQ = 512           # tq block size
TKC = 128          # tk chunk size
NQB = T // TQ      # 4
NKC = T // TKC     # 16
NDCH = D // 128    # 8 contraction chunks over D
VSW = HL * 65 + 64  # staged-V width: 8*[V_h|1] + ones tail pad for M=128 lhsT

# rough per-instruction engine-busy estimates (ns) for the static scheduler
EST_QK = 250       # two concurrent row-tiled K=64 matmuls, N=512
EST_PV = 430       # two K=128 matmuls, N=512
EST_EXP = 1290     # ACT exp on [128, 1024] (measured)
EST_FILL = 215     # one N=512 matmul
RESERVE = 700

_CACHE = {}


def _build(causal: bool, qkv_bias: bool):
    nc = bacc.Bacc("TRN2", target_bir_lowering=False, debug=False,
                   num_devices=N_CORES)
    xT_d = nc.dram_tensor("xT", [D, T], F32, kind="ExternalInput").ap()
    wqk_d = nc.dram_tensor("wqk", [D, 2 * DL], F32, kind="ExternalInput").ap()
    wv_d = nc.dram_tensor("wv", [D, DL], F32, kind="ExternalInput").ap()
    wp_d = nc.dram_tensor("wproj", [DL, D], F32, kind="ExternalInput").ap()
    bqk_d = nc.dram_tensor("bqk", [1, 2 * DL], F32, kind="ExternalInput").ap()
    bv_d = nc.dram_tensor("bv", [1, DL], F32, kind="ExternalInput").ap()
    masks_d = nc.dram_tensor("masks", [TKC, 4 * TQ], BF16,
                             kind="ExternalInput").ap()
    out_d = nc.dram_tensor("out", [T, D], F32, kind="ExternalOutput").ap()

    with tile.TileContext(nc) as tc, ExitStack() as top:
        persist = top.enter_context(tc.tile_pool(name="persist", bufs=1))
        wstage = top.enter_context(tc.tile_pool(name="wstage", bufs=2))
        xstage = top.enter_context(tc.tile_pool(name="xstage", bufs=1))
        xrpool = top.enter_context(
            tc.tile_pool(name="xrpool", bufs=2 if causal else 4))
        ps_s = top.enter_context(tc.tile_pool(name="ps_s", bufs=2, space="PSUM"))
        ps_o = top.enter_context(tc.tile_pool(name="ps_o", bufs=2, space="PSUM"))
        ps_sh = top.enter_context(tc.tile_pool(name="ps_sh", bufs=2, space="PSUM"))
        ppool = top.enter_context(tc.tile_pool(name="ppool", bufs=6))
        npool = top.enter_context(tc.tile_pool(name="npool", bufs=2))
        opool = top.enter_context(tc.tile_pool(name="opool", bufs=3))

        # ---------------- persistent tiles ----------------
        q2 = [persist.tile([128, T], BF16, tag=f"q2{i}", name=f"q2{i}")
              for i in range(4)]       # head-pair packed Q^T
        kT = [persist.tile([128, T], BF16, tag=f"kT{i}", name=f"kT{i}")
              for i in range(4)]       # head-pair packed K^T
        vs = [persist.tile([128, VSW], BF16, tag=f"vs{t}", name=f"vs{t}")
              for t in range(NKC)]     # staged V: [V_h|1]*8 + ones tail
        yT = [persist.tile([128, T], BF16, tag=f"yT{i}", name=f"yT{i}")
              for i in range(4)]
        wqk_r = [persist.tile([128, 2 * DL], BF16, tag=f"wqk{d}", name=f"wqk{d}")
                 for d in range(NDCH)]
        wv_r = [persist.tile([128, DL], BF16, tag=f"wv{d}", name=f"wv{d}")
                for d in range(NDCH)]
        wp_r = [persist.tile([128, D], BF16, tag=f"wp{k}", name=f"wp{k}")
                for k in range(4)]
        ones_r = persist.tile([1, 128], F32R, tag="ones_r", name="ones_r")
        maskb = None
        if causal:
            maskb = persist.tile([TKC, 4 * TQ], BF16, tag="maskb", name="maskb")
            nc.gpsimd.dma_start(maskb[:], masks_d)

        # ---------------- preamble ----------------
        initp = top.enter_context(tc.tile_pool(name="initp", bufs=1))
        ones_f = initp.tile([1, 512], F32, tag="ones_f", name="ones_f")
        nc.vector.memset(ones_f[:], 1.0)
        nc.vector.tensor_copy(ones_r[:], ones_f[:, 0:128])
        bqk_r = bv_r = ones512_r = None
        if qkv_bias:
            ones512_r = initp.tile([1, 512], F32R, tag="ones512_r",
                                   name="ones512_r")
            nc.vector.tensor_copy(ones512_r[:], ones_f[:])
            bqk_f = initp.tile([1, 2 * DL], F32, tag="bqk_f", name="bqk_f")
            nc.gpsimd.dma_start(bqk_f[:], bqk_d)
            bqk_r = initp.tile([1, 2 * DL], F32R, tag="bqk_r", name="bqk_r")
            nc.vector.tensor_copy(bqk_r[:], bqk_f[:])
            bv_f = initp.tile([1, DL], F32, tag="bv_f", name="bv_f")
            nc.gpsimd.dma_start(bv_f[:], bv_d)
            bv_r = initp.tile([1, DL], F32R, tag="bv_r", name="bv_r")
            nc.vector.tensor_copy(bv_r[:], bv_f[:])

        # weights: DMA stage f32 -> DVE cast to bf16 resident copies
        for d in range(NDCH):
            st = wstage.tile([128, 2 * DL], F32, tag="wqks", name=f"wqks{d}")
            nc.gpsimd.dma_start(st[:], wqk_d[d * 128:(d + 1) * 128, :])
            nc.scalar.copy(wqk_r[d][:], st[:])
            stv = wstage.tile([128, DL], F32, tag="wvs", name=f"wvs{d}")
            nc.gpsimd.dma_start(stv[:], wv_d[d * 128:(d + 1) * 128, :])
            nc.scalar.copy(wv_r[d][:], stv[:])
        for k in range(4):
            st = wstage.tile([128, D], F32, tag="wps", name=f"wps{k}")
            nc.gpsimd.dma_start(st[:], wp_d[k * 128:(k + 1) * 128, :])
            nc.gpsimd.tensor_copy(wp_r[k][:], st[:])

        # staged-V tiles start as all-ones; the V copies overwrite the V
        # columns and leave the |1 columns and the tail as ones.
        for t in range(NKC):
            nc.vector.memset(vs[t][:], 1.0)

        # ---------------- x loads (DMA + gpsimd cast) ----------------
        xr_cache = {}

        def xload(j):
            jsl = slice(j * TQ, (j + 1) * TQ)
            xr_j = []
            for d in range(NDCH):
                st = xstage.tile([128, TQ], F32, tag=f"xs{d}", name=f"xs{j}_{d}")
                nc.sync.dma_start(st[:], xT_d[d * 128:(d + 1) * 128, jsl])
                xr_t = xrpool.tile([128, TQ], BF16, tag=f"xr{d}",
                                   name=f"xr{j}_{d}")
                if j < 2:
                    nc.vector.tensor_copy(xr_t[:], st[:])
                else:
                    nc.gpsimd.tensor_copy(xr_t[:], st[:])
                xr_j.append(xr_t)
            xr_cache[j] = xr_j

        # ---------------- filler machinery ----------------
        filler = deque()   # items: (label_or_None, fn, est_pe_ns)
        done = set()
        est = {"pe": 0.0, "act": 0.0}

        def pop_one():
            label, fn, cost = filler.popleft()
            fn()
            if label is not None:
                done.add(label)
            est["pe"] += cost

        def need(label):
            while label not in done:
                assert filler, f"gate {label} not in filler"
                pop_one()

        def budget_pops():
            while filler and est["pe"] + RESERVE < est["act"]:
                pop_one()

        def ph1_steps(j):
            """Phase-1 QKV projection for query block j as filler items."""
            jsl = slice(j * TQ, (j + 1) * TQ)
            xr_j = xr_cache[j]
            items = []

            def m_group(m):
                cell = {}

                def mk(d):
                    def fn():
                        if d == 0:
                            cell["ps"] = ps_sh.tile(
                                [128, TQ], F32, tag="sh", name=f"psqk{j}_{m}")
                        ps = cell["ps"]
                        last = (d == NDCH - 1) and not qkv_bias
                        nc.tensor.matmul(
                            ps[:], wqk_r[d][:, m * 128:(m + 1) * 128],
                            xr_j[d][:], start=(d == 0), stop=last)
                        if d == NDCH - 1:
                            if qkv_bias:
                                nc.tensor.matmul(
                                    ps[:], bqk_r[0:1, m * 128:(m + 1) * 128],
                                    ones512_r[:], start=False, stop=True)
                            dst = q2[m] if m < 4 else kT[m - 4]
                            nc.vector.tensor_copy(dst[:, jsl], ps[:])
                    return fn

                return ([(None, mk(d), EST_FILL) for d in range(NDCH - 1)]
                        + [(("m", j, m), mk(NDCH - 1), EST_FILL + 60)])

            def v_group(tt):
                c = tt % 4
                cell = {}

                def mk(d):
                    def fn():
                        if d == 0:
                            cell["ps"] = ps_sh.tile(
                                [128, DL], F32, tag="sh", name=f"psv{tt}")
                        ps = cell["ps"]
                        last = (d == NDCH - 1) and not qkv_bias
                        nc.tensor.matmul(
                            ps[:], xr_j[d][:, c * 128:(c + 1) * 128],
                            wv_r[d][:], start=(d == 0), stop=last)
                        if d == NDCH - 1:
                            if qkv_bias:
                                nc.tensor.matmul(
                                    ps[:], ones_r[:, 0:128], bv_r[:],
                                    start=False, stop=True)
                            src = ps.rearrange("p (h x) -> p h x", h=HL)
                            dst = vs[tt][:, 0:HL * 65].rearrange(
                                "p (h x) -> p h x", x=65)[:, :, 0:64]
                            nc.vector.tensor_copy(dst, src)
                    return fn

                return ([(None, mk(d), EST_FILL) for d in range(NDCH - 1)]
                        + [(("v", j, tt % 4), mk(NDCH - 1), EST_FILL + 60)])

            # order: what attention needs first -- pair-0 Q/K, then V, then
            # the remaining pairs' Q/K.
            items += m_group(0) + m_group(4)
            for tt in range(4 * j, 4 * j + 4):
                items += v_group(tt)
            for i in range(1, 4):
                items += m_group(i) + m_group(4 + i)
            return items

        def proj_steps(j):
            items = []
            for t in range(4 * j, 4 * j + 4):
                for nb in range(2):
                    def fn(t=t, nb=nb):
                        nsl = slice(nb * 512, (nb + 1) * 512)
                        ps3 = ps_sh.tile([128, TQ], F32, tag="sh",
                                         name=f"ps3_{t}_{nb}")
                        for k in range(4):
                            nc.tensor.matmul(
                                ps3[:], yT[k][:, t * 128:(t + 1) * 128],
                                wp_r[k][:, nsl], start=(k == 0), stop=(k == 3))
                        ot = opool.tile([128, TQ], F32, tag="ot",
                                        name=f"ot{t}_{nb}")
                        nc.vector.tensor_copy(ot[:], ps3[:])
                        nc.sync.dma_start(out_d[t * 128:(t + 1) * 128, nsl],
                                          ot[:])
                    items.append((None, fn, 4 * EST_FILL))
            return items

        norm_q = deque()   # deferred [pb matmul + yT mul] closures
        proj_reserve = []  # proj items held back to fill the last attn block

        def drain_norm():
            while norm_q:
                norm_q.popleft()()
                est["pe"] += 2 * EST_FILL

        # ---------------- main pipelined loop ----------------
        xload(0)
        if causal:
            xload(1)
            filler.extend(ph1_steps(0))
        else:
            for j in range(1, NQB):
                xload(j)
            for j in range(NQB):
                filler.extend(ph1_steps(j))

        for j in range(NQB):
            jsl = slice(j * TQ, (j + 1) * TQ)
            cs = list(range(4 * (j + 1))) if causal else list(range(NKC))
            if causal and j + 2 < NQB:
                xload(j + 2)
            if causal and j + 1 < NQB:
                filler.extend(ph1_steps(j + 1))
            if causal and j == NQB - 1:
                filler.extend(proj_reserve)
                proj_reserve.clear()

            for i in range(4):          # head pair (2i, 2i+1)
                hA, hB = 2 * i, 2 * i + 1
                need(("m", j, i))
                poA = ps_o.tile([128, TQ], F32, tag="po", name=f"poA{j}_{i}")
                poB = ps_o.tile([128, TQ], F32, tag="po", name=f"poB{j}_{i}")

                pend = None   # pipeline: PV(c) emitted after QK(c+1)
                for ci, c in enumerate(cs):
                    need(("m", c // 4, 4 + i))
                    csl = slice(c * TKC, (c + 1) * TKC)
                    ss = ps_s.tile([TKC, 2 * TQ], F32, tag="ss",
                                   name=f"ss{j}_{i}_{c}")
                    nc.tensor.matmul(ss[:, 0:TQ], kT[i][0:64, csl],
                                     q2[i][0:64, jsl], start=True, stop=True)
                    nc.tensor.matmul(ss[:, TQ:2 * TQ], kT[i][64:128, csl],
                                     q2[i][64:128, jsl], start=True, stop=True)
                    est["pe"] += EST_QK
                    pt = ppool.tile([TKC, 2 * TQ], BF16, tag="pt",
                                    name=f"pt{j}_{i}_{c}")
                    nc.scalar.activation(pt[:], ss[:], EXP, scale=0.125)
                    est["act"] += EST_EXP
                    if causal and c >= 4 * j:
                        s = c - 4 * j
                        msl = slice(s * TQ, (s + 1) * TQ)
                        nc.vector.tensor_mul(pt[:, 0:TQ], pt[:, 0:TQ],
                                             maskb[:, msl])
                        nc.gpsimd.tensor_mul(pt[:, TQ:2 * TQ],
                                             pt[:, TQ:2 * TQ], maskb[:, msl])
                    if ci == 0:
                        drain_norm()   # previous pair's deferred norm
                    if pend is not None:
                        pc, ppt = pend
                        need(("v", pc // 4, pc % 4))
                        st = (ci == 1)
                        nc.tensor.matmul(
                            poA[:], vs[pc][:, hA * 65:hA * 65 + 128],
                            ppt[:, 0:TQ], start=st, stop=False)
                        nc.tensor.matmul(
                            poB[:], vs[pc][:, hB * 65:hB * 65 + 128],
                            ppt[:, TQ:2 * TQ], start=st, stop=False)
                        est["pe"] += EST_PV
                    pend = (c, pt)
                    budget_pops()
                pc, ppt = pend
                need(("v", pc // 4, pc % 4))
                one = (len(cs) == 1)
                nc.tensor.matmul(poA[:], vs[pc][:, hA * 65:hA * 65 + 128],
                                 ppt[:, 0:TQ], start=one, stop=True)
                nc.tensor.matmul(poB[:], vs[pc][:, hB * 65:hB * 65 + 128],
                                 ppt[:, TQ:2 * TQ], start=one, stop=True)
                est["pe"] += EST_PV

                # immediate DVE part of softmax normalization
                rr, osb = [], []
                for h, po in ((hA, poA), (hB, poB)):
                    sums = npool.tile([1, TQ], F32, tag="sums",
                                      name=f"sm{j}_{h}")
                    nc.vector.tensor_copy(sums[:], po[64:65, :])
                    o_sb = npool.tile([64, TQ], BF16, tag="o_sb",
                                      name=f"ob{j}_{h}")
                    nc.vector.tensor_copy(o_sb[:], po[0:64, :])
                    recip = npool.tile([1, TQ], F32, tag="recip",
                                       name=f"rc{j}_{h}")
                    recip_r = npool.tile([1, TQ], F32R, tag="recip_r",
                                         name=f"rr{j}_{h}")
                    nc.vector.reciprocal_approx_fast(
                        out=recip[:], in_=sums[:])
                    nc.vector.tensor_copy(recip_r[:], recip[:])
                    rr.append(recip_r)
                    osb.append(o_sb)

                def norm_fn(i=i, jsl=jsl, osb=osb, rr=rr, j=j, hA=hA):
                    for hp in (0, 1):
                        pb = ps_sh.tile([64, TQ], F32, tag="sh",
                                        name=f"pb{j}_{hA + hp}")
                        nc.tensor.matmul(pb[:], ones_r[:, 0:64], rr[hp][:],
                                         start=True, stop=True)
                        nc.vector.tensor_mul(
                            yT[i][hp * 64:(hp + 1) * 64, jsl],
                            osb[hp][:], pb[:])
                norm_q.append(norm_fn)

            drain_norm()               # before proj(j) can be emitted
            items = proj_steps(j)
            if causal and j < 2:       # hold some proj back to fill attn(3)
                filler.extend(items[:4])
                proj_reserve.extend(items[4:])
            else:
                filler.extend(items)

        while filler:                  # flush remaining projection work
            pop_one()

    nc.compile()
    return nc


def _get_nc(causal: bool, qkv_bias: bool = False):
    key = (causal, qkv_bias)
    if key not in _CACHE:
        _CACHE[key] = _build(causal, qkv_bias)
    return _CACHE[key]


def _host_masks() -> np.ndarray:
    i = np.arange(TKC)[:, None]
    jj = np.arange(TQ)[None, :]
    blocks = [(jj >= i + s * TKC).astype(np.float32) for s in range(4)]
    return np.ascontiguousarray(
        np.concatenate(blocks, axis=1).astype(ml_dtypes.bfloat16))


def _make_in_maps(x, W_qkv, b_qkv, W_proj):
    masks_np = _host_masks()
    in_maps = []
    for core in range(N_CORES):
        b, g = core // 2, core % 2
        qc = slice(g * DL, (g + 1) * DL)
        kc = slice(D + g * DL, D + (g + 1) * DL)
        vc = slice(2 * D + g * DL, 2 * D + (g + 1) * DL)
        in_maps.append({
            "xT": np.ascontiguousarray(x[b].T),
            "wqk": np.ascontiguousarray(
                np.concatenate([W_qkv[:, qc], W_qkv[:, kc]], axis=1)),
            "wv": np.ascontiguousarray(W_qkv[:, vc]),
            "bqk": np.ascontiguousarray(
                np.concatenate([b_qkv[qc], b_qkv[kc]]).reshape(1, 2 * DL)),
            "bv": np.ascontiguousarray(b_qkv[vc].reshape(1, DL)),
            "wproj": np.ascontiguousarray(W_proj[g * DL:(g + 1) * DL, :]),
            "masks": masks_np,
        })
    return in_maps


def kernel(x, mask, W_qkv, b_qkv, W_proj, b_proj):
    x = np.asarray(x, dtype=np.float32)
    mask2d = np.asarray(mask, dtype=np.int32).reshape(T, T)
    W_qkv = np.asarray(W_qkv, dtype=np.float32)
    b_qkv = np.asarray(b_qkv, dtype=np.float32)
    W_proj = np.asarray(W_proj, dtype=np.float32)
    b_proj = np.asarray(b_proj, dtype=np.float32)

    if np.array_equal(mask2d, np.tril(np.ones((T, T), dtype=np.int32))):
        causal = True
    elif np.all(mask2d == 1):
        causal = False
    else:
        raise NotImplementedError("only causal (tril) or all-ones masks")

    qkv_bias = bool(np.any(b_qkv != 0.0))
    nc = _get_nc(causal, qkv_bias)
    in_maps = _make_in_maps(x, W_qkv, b_qkv, W_proj)
    res = run_bass_kernel_spmd(nc, in_maps, core_ids=list(range(N_CORES)))
    out = np.empty((B, T, D), dtype=np.float32)
    for b in range(B):
        out[b] = (res.results[2 * b]["out"] + res.results[2 * b + 1]["out"]
                  + b_proj[None, :])
    return out


# revision 7
# speedup vs baseline: 1.2937x; 1.0621x over previous
"""Multi-head causal self-attention for TRN2, 8 NeuronCores.

Sharding: core i handles (batch b = i//2, head-group g = i%2); each head-group
is 8 of the 16 heads.  Per core everything is computed in "transposed" space so
no on-device transposes are needed.

v2 vs baseline:
  * QK attention matmuls are row-tiled: the two heads of a pair run as
    concurrent K=64 matmuls on PE row-groups (0,0)/(64,0) writing the two
    halves (separate PSUM banks) of one [128, 1024] score tile, so both
    heads' S^T cost one 512-col stream instead of two.
  * Single software-pipelined loop: phase-1 QKV projection work for block
    j+1 and the output projection for block j-1 are emitted as fine-grained
    "filler" matmuls inside the ACT(exp)-paced attention chunk loop, with
    FIFO gating so attention never waits on un-emitted producers.
  * No ACT in phase 1: QKV biases (when nonzero) are rank-1 matmuls; PSUM
    drains are DVE copies.  ACT does only the 160 softmax exp calls.
  * x^T f32->bf16 casts run on the otherwise-idle GPSIMD engine.
  * softmax normalization: reciprocal on DVE, [1,q]->[64,q] broadcast via a
    K=1 matmul into the shared PSUM pool, deferred past the next pair's
    first chunk so the PE never stalls on the DVE reciprocal chain.
"""

import numpy as np
import ml_dtypes
from collections import deque
from contextlib import ExitStack

import concourse.bass as bass
import concourse.mybir as mybir
import concourse.tile as tile
from concourse import bacc
from concourse.bass_utils import run_bass_kernel_spmd

B, T, D, H = 4, 2048, 1024, 16
DK = 64            # head dim
HL = 8             # heads per core
DL = HL * DK       # 512 local head dims per core
N_CORES = 8

F32 = mybir.dt.float32
F32R = mybir.dt.float32r
BF16 = mybir.dt.bfloat16
EXP = mybir.ActivationFunctionType.Exp

TQ = 512           # tq block size
TKC = 128          # tk chunk size
NQB = T // TQ      # 4
NKC = T // TKC     # 16
NDCH = D // 128    # 8 contraction chunks over D
VSW = HL * 65 + 64  # staged-V width: 8*[V_h|1] + ones tail pad for M=128 lhsT

# rough per-instruction engine-busy estimates (ns) for the static scheduler
EST_QK = 350       # two concurrent row-tiled K=64 matmuls, N=512
EST_PV = 560       # two K=128 matmuls, N=512
EST_EXP = 1080     # ACT exp on [128, 1024] (measured)
EST_FILL = 300     # one N=512 matmul
RESERVE = 800

_CACHE = {}


def _build(causal: bool, qkv_bias: bool):
    nc = bacc.Bacc("TRN2", target_bir_lowering=False, debug=False,
                   num_devices=N_CORES)
    xT_d = nc.dram_tensor("xT", [D, T], F32, kind="ExternalInput").ap()
    wqk_d = nc.dram_tensor("wqk", [D, 2 * DL], F32, kind="ExternalInput").ap()
    wv_d = nc.dram_tensor("wv", [D, DL], F32, kind="ExternalInput").ap()
    wp_d = nc.dram_tensor("wproj", [DL, D], F32, kind="ExternalInput").ap()
    bqk_d = nc.dram_tensor("bqk", [1, 2 * DL], F32, kind="ExternalInput").ap()
    bv_d = nc.dram_tensor("bv", [1, DL], F32, kind="ExternalInput").ap()
    masks_d = nc.dram_tensor("masks", [TKC, TKC], BF16,
                             kind="ExternalInput").ap()
    out_d = nc.dram_tensor("out", [T, D], F32, kind="ExternalOutput").ap()

    with tile.TileContext(nc) as tc, ExitStack() as top:
        persist = top.enter_context(tc.tile_pool(name="persist", bufs=1))
        wstage = top.enter_context(tc.tile_pool(name="wstage", bufs=2))
        xstage = top.enter_context(tc.tile_pool(name="xstage", bufs=1))
        xrpool = top.enter_context(
            tc.tile_pool(name="xrpool", bufs=2 if causal else 4))
        ps_s = top.enter_context(tc.tile_pool(name="ps_s", bufs=2, space="PSUM"))
        ps_o = top.enter_context(tc.tile_pool(name="ps_o", bufs=2, space="PSUM"))
        ps_sh = top.enter_context(tc.tile_pool(name="ps_sh", bufs=2, space="PSUM"))
        ppool = top.enter_context(tc.tile_pool(name="ppool", bufs=6))
        npool = top.enter_context(tc.tile_pool(name="npool", bufs=2))
        opool = top.enter_context(tc.tile_pool(name="opool", bufs=3))

        # ---------------- persistent tiles ----------------
        q2 = [persist.tile([128, T], BF16, tag=f"q2{i}", name=f"q2{i}")
              for i in range(4)]       # head-pair packed Q^T
        kT = [persist.tile([128, T], BF16, tag=f"kT{i}", name=f"kT{i}")
              for i in range(4)]       # head-pair packed K^T
        vs = [persist.tile([128, VSW], BF16, tag=f"vs{t}", name=f"vs{t}")
              for t in range(NKC)]     # staged V: [V_h|1]*8 + ones tail
        yT = [persist.tile([128, T], BF16, tag=f"yT{i}", name=f"yT{i}")
              for i in range(4)]
        wqk_r = [persist.tile([128, 2 * DL], BF16, tag=f"wqk{d}", name=f"wqk{d}")
                 for d in range(NDCH)]
        wv_r = [persist.tile([128, DL], BF16, tag=f"wv{d}", name=f"wv{d}")
                for d in range(NDCH)]
        wp_r = [persist.tile([128, D], BF16, tag=f"wp{k}", name=f"wp{k}")
                for k in range(4)]
        ones_r = persist.tile([1, 128], F32R, tag="ones_r", name="ones_r")
        maskb = None
        if causal:
            maskb = persist.tile([TKC, TKC], BF16, tag="maskb", name="maskb")
            nc.gpsimd.dma_start(maskb[:], masks_d)

        # ---------------- preamble ----------------
        initp = top.enter_context(tc.tile_pool(name="initp", bufs=1))
        ones_f = initp.tile([1, 512], F32, tag="ones_f", name="ones_f")
        nc.vector.memset(ones_f[:], 1.0)
        nc.vector.tensor_copy(ones_r[:], ones_f[:, 0:128])
        bqk_r = bv_r = ones512_r = None
        if qkv_bias:
            ones512_r = initp.tile([1, 512], F32R, tag="ones512_r",
                                   name="ones512_r")
            nc.vector.tensor_copy(ones512_r[:], ones_f[:])
            bqk_f = initp.tile([1, 2 * DL], F32, tag="bqk_f", name="bqk_f")
            nc.gpsimd.dma_start(bqk_f[:], bqk_d)
            bqk_r = initp.tile([1, 2 * DL], F32R, tag="bqk_r", name="bqk_r")
            nc.vector.tensor_copy(bqk_r[:], bqk_f[:])
            bv_f = initp.tile([1, DL], F32, tag="bv_f", name="bv_f")
            nc.gpsimd.dma_start(bv_f[:], bv_d)
            bv_r = initp.tile([1, DL], F32R, tag="bv_r", name="bv_r")
            nc.vector.tensor_copy(bv_r[:], bv_f[:])

        # weights: DMA stage f32 -> DVE cast to bf16 resident copies
        dmaq = [nc.gpsimd, nc.scalar]
        for d in range(NDCH):
            st = wstage.tile([128, 2 * DL], F32, tag="wqks", name=f"wqks{d}")
            dmaq[d % 2].dma_start(st[:], wqk_d[d * 128:(d + 1) * 128, :])
            nc.scalar.copy(wqk_r[d][:], st[:])
        for d in range(NDCH):
            stv = wstage.tile([128, DL], F32, tag="wvs", name=f"wvs{d}")
            dmaq[d % 2].dma_start(stv[:], wv_d[d * 128:(d + 1) * 128, :])
            nc.scalar.copy(wv_r[d][:], stv[:])
        for k in range(4):
            st = wstage.tile([128, D], F32, tag="wps", name=f"wps{k}")
            dmaq[k % 2].dma_start(st[:], wp_d[k * 128:(k + 1) * 128, :])
            nc.gpsimd.tensor_copy(wp_r[k][:], st[:])

        # staged-V tiles start as all-ones; the V copies overwrite the V
        # columns and leave the |1 columns and the tail as ones.
        for t in range(NKC):
            nc.vector.memset(vs[t][:], 1.0)

        # ---------------- x loads (DMA + gpsimd cast) ----------------
        xr_cache = {}

        def xload(j):
            jsl = slice(j * TQ, (j + 1) * TQ)
            xr_j = []
            for d in range(NDCH):
                st = xstage.tile([128, TQ], F32, tag=f"xs{d}", name=f"xs{j}_{d}")
                nc.sync.dma_start(st[:], xT_d[d * 128:(d + 1) * 128, jsl])
                xr_t = xrpool.tile([128, TQ], BF16, tag=f"xr{d}",
                                   name=f"xr{j}_{d}")
                if j < 2:
                    nc.vector.tensor_copy(xr_t[:], st[:])
                else:
                    nc.gpsimd.tensor_copy(xr_t[:], st[:])
                xr_j.append(xr_t)
            xr_cache[j] = xr_j

        # ---------------- filler machinery ----------------
        filler = deque()   # items: (label_or_None, fn, est_pe_ns)
        done = set()
        est = {"pe": 0.0, "act": 0.0}

        def pop_one():
            label, fn, cost = filler.popleft()
            fn()
            if label is not None:
                done.add(label)
            est["pe"] += cost

        def need(label):
            while label not in done:
                assert filler, f"gate {label} not in filler"
                pop_one()

        def budget_pops():
            while filler and est["pe"] + RESERVE < est["act"]:
                pop_one()

        def ph1_steps(j):
            """Phase-1 QKV projection for query block j as filler items."""
            jsl = slice(j * TQ, (j + 1) * TQ)
            xr_j = xr_cache[j]
            items = []

            def m_group(m):
                cell = {}

                def mk(d):
                    def fn():
                        if d == 0:
                            cell["ps"] = ps_sh.tile(
                                [128, TQ], F32, tag="sh", name=f"psqk{j}_{m}")
                        ps = cell["ps"]
                        last = (d == NDCH - 1) and not qkv_bias
                        nc.tensor.matmul(
                            ps[:], wqk_r[d][:, m * 128:(m + 1) * 128],
                            xr_j[d][:], start=(d == 0), stop=last)
                        if d == NDCH - 1:
                            if qkv_bias:
                                nc.tensor.matmul(
                                    ps[:], bqk_r[0:1, m * 128:(m + 1) * 128],
                                    ones512_r[:], start=False, stop=True)
                            dst = q2[m] if m < 4 else kT[m - 4]
                            nc.vector.tensor_copy(dst[:, jsl], ps[:])
                    return fn

                return ([(None, mk(d), EST_FILL) for d in range(NDCH - 1)]
                        + [(("m", j, m), mk(NDCH - 1), EST_FILL + 60)])

            def v_group(tt):
                c = tt % 4
                cell = {}

                def mk(d):
                    def fn():
                        if d == 0:
                            cell["ps"] = ps_sh.tile(
                                [128, DL], F32, tag="sh", name=f"psv{tt}")
                        ps = cell["ps"]
                        last = (d == NDCH - 1) and not qkv_bias
                        nc.tensor.matmul(
                            ps[:], xr_j[d][:, c * 128:(c + 1) * 128],
                            wv_r[d][:], start=(d == 0), stop=last)
                        if d == NDCH - 1:
                            if qkv_bias:
                                nc.tensor.matmul(
                                    ps[:], ones_r[:, 0:128], bv_r[:],
                                    start=False, stop=True)
                            src = ps.rearrange("p (h x) -> p h x", h=HL)
                            dst = vs[tt][:, 0:HL * 65].rearrange(
                                "p (h x) -> p h x", x=65)[:, :, 0:64]
                            nc.vector.tensor_copy(dst, src)
                    return fn

                return ([(None, mk(d), EST_FILL) for d in range(NDCH - 1)]
                        + [(("v", j, tt % 4), mk(NDCH - 1), EST_FILL + 60)])

            # order: what attention needs first -- pair-0 Q/K, then V, then
            # the remaining pairs' Q/K.
            items += m_group(0) + m_group(4)
            for tt in range(4 * j, 4 * j + 4):
                items += v_group(tt)
            for i in range(1, 4):
                items += m_group(i) + m_group(4 + i)
            return items

        def proj_steps(j):
            items = []
            for t in range(4 * j, 4 * j + 4):
                for nb in range(2):
                    def fn(t=t, nb=nb):
                        nsl = slice(nb * 512, (nb + 1) * 512)
                        ps3 = ps_sh.tile([128, TQ], F32, tag="sh",
                                         name=f"ps3_{t}_{nb}")
                        for k in range(4):
                            nc.tensor.matmul(
                                ps3[:], yT[k][:, t * 128:(t + 1) * 128],
                                wp_r[k][:, nsl], start=(k == 0), stop=(k == 3))
                        ot = opool.tile([128, TQ], F32, tag="ot",
                                        name=f"ot{t}_{nb}")
                        nc.vector.tensor_copy(ot[:], ps3[:])
                        nc.sync.dma_start(out_d[t * 128:(t + 1) * 128, nsl],
                                          ot[:])
                    items.append((None, fn, 4 * EST_FILL))
            return items

        norm_q = deque()   # deferred [pb matmul + yT mul] closures
        proj_reserve = []  # proj items held back to fill the last attn block

        def drain_norm():
            while norm_q:
                norm_q.popleft()()
                est["pe"] += 2 * EST_FILL

        # ---------------- main pipelined loop ----------------
        xload(0)
        if causal:
            xload(1)
            filler.extend(ph1_steps(0))
        else:
            for j in range(1, NQB):
                xload(j)
            for j in range(NQB):
                filler.extend(ph1_steps(j))

        for j in range(NQB):
            jsl = slice(j * TQ, (j + 1) * TQ)
            cs = list(range(4 * (j + 1))) if causal else list(range(NKC))
            if causal and j + 2 < NQB:
                xload(j + 2)
            if causal and j + 1 < NQB:
                filler.extend(ph1_steps(j + 1))
            if causal and j == NQB - 1:
                filler.extend(proj_reserve)
                proj_reserve.clear()

            for i in range(4):          # head pair (2i, 2i+1)
                hA, hB = 2 * i, 2 * i + 1
                need(("m", j, i))
                poA = ps_o.tile([128, TQ], F32, tag="po", name=f"poA{j}_{i}")
                poB = ps_o.tile([128, TQ], F32, tag="po", name=f"poB{j}_{i}")

                pend = None   # pipeline: PV(c) emitted after QK(c+1)
                def qskip(c):
                    # fully-masked leading query columns of a diagonal chunk
                    if causal and c >= 4 * j:
                        return (c - 4 * j) * TKC
                    return 0

                def pv_emit(pc, ppt, start, stop):
                    k0 = qskip(pc)
                    nc.tensor.matmul(
                        poA[:, k0:TQ], vs[pc][:, hA * 65:hA * 65 + 128],
                        ppt[:, k0:TQ], start=start, stop=stop)
                    nc.tensor.matmul(
                        poB[:, k0:TQ], vs[pc][:, hB * 65:hB * 65 + 128],
                        ppt[:, TQ + k0:2 * TQ], start=start, stop=stop)
                    est["pe"] += EST_PV

                for ci, c in enumerate(cs):
                    need(("m", c // 4, 4 + i))
                    csl = slice(c * TKC, (c + 1) * TKC)
                    k0 = qskip(c)
                    ss = ps_s.tile([TKC, 2 * TQ], F32, tag="ss",
                                   name=f"ss{j}_{i}_{c}")
                    nc.tensor.matmul(ss[:, k0:TQ], kT[i][0:64, csl],
                                     q2[i][0:64, j * TQ + k0:(j + 1) * TQ],
                                     start=True, stop=True)
                    nc.tensor.matmul(ss[:, TQ + k0:2 * TQ], kT[i][64:128, csl],
                                     q2[i][64:128, j * TQ + k0:(j + 1) * TQ],
                                     start=True, stop=True)
                    est["pe"] += EST_QK
                    pt = ppool.tile([TKC, 2 * TQ], BF16, tag="pt",
                                    name=f"pt{j}_{i}_{c}")
                    nc.scalar.activation(pt[:], ss[:], EXP, scale=0.125)
                    est["act"] += EST_EXP
                    if causal and c >= 4 * j:
                        # only the 128-wide diagonal band is partially masked
                        bsl = slice(k0, k0 + TKC)
                        nc.vector.tensor_mul(pt[:, bsl], pt[:, bsl], maskb[:])
                        bslB = slice(TQ + k0, TQ + k0 + TKC)
                        nc.vector.tensor_mul(pt[:, bslB], pt[:, bslB],
                                             maskb[:])
                    if ci == 0:
                        drain_norm()   # previous pair's deferred norm
                    if pend is not None:
                        pc, ppt = pend
                        need(("v", pc // 4, pc % 4))
                        pv_emit(pc, ppt, ci == 1, False)
                    pend = (c, pt)
                    budget_pops()
                pc, ppt = pend
                need(("v", pc // 4, pc % 4))
                pv_emit(pc, ppt, len(cs) == 1, True)

                # immediate DVE part of softmax normalization
                rr, osb = [], []
                for h, po in ((hA, poA), (hB, poB)):
                    sums = npool.tile([1, TQ], F32, tag="sums",
                                      name=f"sm{j}_{h}")
                    nc.vector.tensor_copy(sums[:], po[64:65, :])
                    o_sb = npool.tile([64, TQ], BF16, tag="o_sb",
                                      name=f"ob{j}_{h}")
                    nc.vector.tensor_copy(o_sb[:], po[0:64, :])
                    recip = npool.tile([1, TQ], F32, tag="recip",
                                       name=f"rc{j}_{h}")
                    recip_r = npool.tile([1, TQ], F32R, tag="recip_r",
                                         name=f"rr{j}_{h}")
                    nc.vector.reciprocal_approx_fast(
                        out=recip[:], in_=sums[:])
                    nc.vector.tensor_copy(recip_r[:], recip[:])
                    rr.append(recip_r)
                    osb.append(o_sb)

                def norm_fn(i=i, jsl=jsl, osb=osb, rr=rr, j=j, hA=hA):
                    for hp in (0, 1):
                        pb = ps_sh.tile([64, TQ], F32, tag="sh",
                                        name=f"pb{j}_{hA + hp}")
                        nc.tensor.matmul(pb[:], ones_r[:, 0:64], rr[hp][:],
                                         start=True, stop=True)
                        nc.vector.tensor_mul(
                            yT[i][hp * 64:(hp + 1) * 64, jsl],
                            osb[hp][:], pb[:])
                norm_q.append(norm_fn)

            drain_norm()               # before proj(j) can be emitted
            items = proj_steps(j)
            if causal and j < 2:       # hold some proj back to fill attn(3)
                filler.extend(items[:4])
                proj_reserve.extend(items[4:])
            else:
                filler.extend(items)

        while filler:                  # flush remaining projection work
            pop_one()

    nc.compile()
    return nc


def _get_nc(causal: bool, qkv_bias: bool = False):
    key = (causal, qkv_bias)
    if key not in _CACHE:
        _CACHE[key] = _build(causal, qkv_bias)
    return _CACHE[key]


def _host_masks() -> np.ndarray:
    i = np.arange(TKC)[:, None]
    jj = np.arange(TKC)[None, :]
    return np.ascontiguousarray(
        (jj >= i).astype(np.float32).astype(ml_dtypes.bfloat16))


def _make_in_maps(x, W_qkv, b_qkv, W_proj):
    masks_np = _host_masks()
    in_maps = []
    for core in range(N_CORES):
        b, g = core // 2, core % 2
        qc = slice(g * DL, (g + 1) * DL)
        kc = slice(D + g * DL, D + (g + 1) * DL)
        vc = slice(2 * D + g * DL, 2 * D + (g + 1) * DL)
        in_maps.append({
            "xT": np.ascontiguousarray(x[b].T),
            "wqk": np.ascontiguousarray(
                np.concatenate([W_qkv[:, qc], W_qkv[:, kc]], axis=1)),
            "wv": np.ascontiguousarray(W_qkv[:, vc]),
            "bqk": np.ascontiguousarray(
                np.concatenate([b_qkv[qc], b_qkv[kc]]).reshape(1, 2 * DL)),
            "bv": np.ascontiguousarray(b_qkv[vc].reshape(1, DL)),
            "wproj": np.ascontiguousarray(W_proj[g * DL:(g + 1) * DL, :]),
            "masks": masks_np,
        })
    return in_maps


def kernel(x, mask, W_qkv, b_qkv, W_proj, b_proj):
    x = np.asarray(x, dtype=np.float32)
    mask2d = np.asarray(mask, dtype=np.int32).reshape(T, T)
    W_qkv = np.asarray(W_qkv, dtype=np.float32)
    b_qkv = np.asarray(b_qkv, dtype=np.float32)
    W_proj = np.asarray(W_proj, dtype=np.float32)
    b_proj = np.asarray(b_proj, dtype=np.float32)

    if np.array_equal(mask2d, np.tril(np.ones((T, T), dtype=np.int32))):
        causal = True
    elif np.all(mask2d == 1):
        causal = False
    else:
        raise NotImplementedError("only causal (tril) or all-ones masks")

    qkv_bias = bool(np.any(b_qkv != 0.0))
    nc = _get_nc(causal, qkv_bias)
    in_maps = _make_in_maps(x, W_qkv, b_qkv, W_proj)
    res = run_bass_kernel_spmd(nc, in_maps, core_ids=list(range(N_CORES)))
    out = np.empty((B, T, D), dtype=np.float32)
    for b in range(B):
        out[b] = (res.results[2 * b]["out"] + res.results[2 * b + 1]["out"]
                  + b_proj[None, :])
    return out


# revision 8
# speedup vs baseline: 1.3246x; 1.0239x over previous
"""Multi-head causal self-attention for TRN2, 8 NeuronCores.

Sharding: core i handles (batch b = i//2, head-group g = i%2); each head-group
is 8 of the 16 heads.  Per core everything is computed in "transposed" space so
no on-device transposes are needed.

v2 vs baseline:
  * QK attention matmuls are row-tiled: the two heads of a pair run as
    concurrent K=64 matmuls on PE row-groups (0,0)/(64,0) writing the two
    halves (separate PSUM banks) of one [128, 1024] score tile, so both
    heads' S^T cost one 512-col stream instead of two.
  * Single software-pipelined loop: phase-1 QKV projection work for block
    j+1 and the output projection for block j-1 are emitted as fine-grained
    "filler" matmuls inside the ACT(exp)-paced attention chunk loop, with
    FIFO gating so attention never waits on un-emitted producers.
  * No ACT in phase 1: QKV biases (when nonzero) are rank-1 matmuls; PSUM
    drains are DVE copies.  ACT does only the 160 softmax exp calls.
  * x^T f32->bf16 casts run on the otherwise-idle GPSIMD engine.
  * softmax normalization: reciprocal on DVE, [1,q]->[64,q] broadcast via a
    K=1 matmul into the shared PSUM pool, deferred past the next pair's
    first chunk so the PE never stalls on the DVE reciprocal chain.
"""

import numpy as np
import ml_dtypes
from collections import deque
from contextlib import ExitStack

import concourse.bass as bass
import concourse.mybir as mybir
import concourse.tile as tile
from concourse import bacc
from concourse.bass_utils import run_bass_kernel_spmd

B, T, D, H = 4, 2048, 1024, 16
DK = 64            # head dim
HL = 8             # heads per core
DL = HL * DK       # 512 local head dims per core
N_CORES = 8

F32 = mybir.dt.float32
F32R = mybir.dt.float32r
BF16 = mybir.dt.bfloat16
EXP = mybir.ActivationFunctionType.Exp

TQ = 512           # tq block size
TKC = 128          # tk chunk size
NQB = T // TQ      # 4
NKC = T // TKC     # 16
NDCH = D // 128    # 8 contraction chunks over D
VSW = HL * 65 + 64  # staged-V width: 8*[V_h|1] + ones tail pad for M=128 lhsT

# rough per-instruction engine-busy estimates (ns) for the static scheduler
EST_QK = 350       # two concurrent row-tiled K=64 matmuls, N=512
EST_PV = 560       # two K=128 matmuls, N=512
EST_EXP = 1080     # ACT exp on [128, 1024] (measured)
EST_FILL = 300     # one N=512 matmul
RESERVE = 800

_CACHE = {}


def _build(causal: bool, qkv_bias: bool):
    nc = bacc.Bacc("TRN2", target_bir_lowering=False, debug=False,
                   num_devices=N_CORES)
    xT_d = nc.dram_tensor("xT", [D, T], F32, kind="ExternalInput").ap()
    wqk_d = nc.dram_tensor("wqk", [D, 2 * DL], F32, kind="ExternalInput").ap()
    wv_d = nc.dram_tensor("wv", [D, DL], F32, kind="ExternalInput").ap()
    wp_d = nc.dram_tensor("wproj", [DL, D], F32, kind="ExternalInput").ap()
    bqk_d = nc.dram_tensor("bqk", [1, 2 * DL], F32, kind="ExternalInput").ap()
    bv_d = nc.dram_tensor("bv", [1, DL], F32, kind="ExternalInput").ap()
    masks_d = nc.dram_tensor("masks", [TKC, TKC], BF16,
                             kind="ExternalInput").ap()
    out_d = nc.dram_tensor("out", [T, D], F32, kind="ExternalOutput").ap()

    with tile.TileContext(nc) as tc, ExitStack() as top:
        persist = top.enter_context(tc.tile_pool(name="persist", bufs=1))
        wstage = top.enter_context(tc.tile_pool(name="wstage", bufs=2))
        xstage = top.enter_context(tc.tile_pool(name="xstage", bufs=1))
        xrpool = top.enter_context(
            tc.tile_pool(name="xrpool", bufs=2 if causal else 4))
        ps_s = top.enter_context(tc.tile_pool(name="ps_s", bufs=2, space="PSUM"))
        ps_o = top.enter_context(tc.tile_pool(name="ps_o", bufs=2, space="PSUM"))
        ps_sh = top.enter_context(tc.tile_pool(name="ps_sh", bufs=2, space="PSUM"))
        ppool = top.enter_context(tc.tile_pool(name="ppool", bufs=8))
        npool = top.enter_context(tc.tile_pool(name="npool", bufs=2))
        opool = top.enter_context(tc.tile_pool(name="opool", bufs=3))

        # ---------------- persistent tiles ----------------
        q2 = [persist.tile([128, T], BF16, tag=f"q2{i}", name=f"q2{i}")
              for i in range(4)]       # head-pair packed Q^T
        kT = [persist.tile([128, T], BF16, tag=f"kT{i}", name=f"kT{i}")
              for i in range(4)]       # head-pair packed K^T
        vs = [persist.tile([128, VSW], BF16, tag=f"vs{t}", name=f"vs{t}")
              for t in range(NKC)]     # staged V: [V_h|1]*8 + ones tail
        yT = [persist.tile([128, T], BF16, tag=f"yT{i}", name=f"yT{i}")
              for i in range(4)]
        wqk_r = [persist.tile([128, 2 * DL], BF16, tag=f"wqk{d}", name=f"wqk{d}")
                 for d in range(NDCH)]
        wv_r = [persist.tile([128, DL], BF16, tag=f"wv{d}", name=f"wv{d}")
                for d in range(NDCH)]
        wp_r = [persist.tile([128, D], BF16, tag=f"wp{k}", name=f"wp{k}")
                for k in range(4)]
        ones_r = persist.tile([1, 128], F32R, tag="ones_r", name="ones_r")
        maskb = None
        if causal:
            maskb = persist.tile([TKC, TKC], BF16, tag="maskb", name="maskb")
            nc.gpsimd.dma_start(maskb[:], masks_d)

        # ---------------- preamble ----------------
        initp = top.enter_context(tc.tile_pool(name="initp", bufs=1))
        ones_f = initp.tile([1, 512], F32, tag="ones_f", name="ones_f")
        nc.vector.memset(ones_f[:], 1.0)
        nc.vector.tensor_copy(ones_r[:], ones_f[:, 0:128])
        bqk_r = bv_r = ones512_r = None
        if qkv_bias:
            ones512_r = initp.tile([1, 512], F32R, tag="ones512_r",
                                   name="ones512_r")
            nc.vector.tensor_copy(ones512_r[:], ones_f[:])
            bqk_f = initp.tile([1, 2 * DL], F32, tag="bqk_f", name="bqk_f")
            nc.gpsimd.dma_start(bqk_f[:], bqk_d)
            bqk_r = initp.tile([1, 2 * DL], F32R, tag="bqk_r", name="bqk_r")
            nc.vector.tensor_copy(bqk_r[:], bqk_f[:])
            bv_f = initp.tile([1, DL], F32, tag="bv_f", name="bv_f")
            nc.gpsimd.dma_start(bv_f[:], bv_d)
            bv_r = initp.tile([1, DL], F32R, tag="bv_r", name="bv_r")
            nc.vector.tensor_copy(bv_r[:], bv_f[:])

        # weights: DMA stage f32 -> DVE cast to bf16 resident copies
        dmaq = [nc.gpsimd, nc.scalar]
        for d in range(NDCH):
            st = wstage.tile([128, 2 * DL], F32, tag="wqks", name=f"wqks{d}")
            dmaq[d % 2].dma_start(st[:], wqk_d[d * 128:(d + 1) * 128, :])
            nc.scalar.copy(wqk_r[d][:], st[:])
        for d in range(NDCH):
            stv = wstage.tile([128, DL], F32, tag="wvs", name=f"wvs{d}")
            dmaq[d % 2].dma_start(stv[:], wv_d[d * 128:(d + 1) * 128, :])
            nc.scalar.copy(wv_r[d][:], stv[:])
        for k in range(4):
            st = wstage.tile([128, D], F32, tag="wps", name=f"wps{k}")
            dmaq[k % 2].dma_start(st[:], wp_d[k * 128:(k + 1) * 128, :])
            nc.gpsimd.tensor_copy(wp_r[k][:], st[:])

        # staged-V tiles start as all-ones; the V copies overwrite the V
        # columns and leave the |1 columns and the tail as ones.
        for t in range(NKC):
            nc.vector.memset(vs[t][:], 1.0)

        # ---------------- x loads (DMA + gpsimd cast) ----------------
        xr_cache = {}

        def xload(j):
            jsl = slice(j * TQ, (j + 1) * TQ)
            xr_j = []
            for d in range(NDCH):
                st = xstage.tile([128, TQ], F32, tag=f"xs{d}", name=f"xs{j}_{d}")
                nc.sync.dma_start(st[:], xT_d[d * 128:(d + 1) * 128, jsl])
                xr_t = xrpool.tile([128, TQ], BF16, tag=f"xr{d}",
                                   name=f"xr{j}_{d}")
                if j < 2:
                    nc.vector.tensor_copy(xr_t[:], st[:])
                else:
                    nc.gpsimd.tensor_copy(xr_t[:], st[:])
                xr_j.append(xr_t)
            xr_cache[j] = xr_j

        # ---------------- filler machinery ----------------
        filler = deque()   # items: (label_or_None, fn, est_pe_ns)
        done = set()
        est = {"pe": 0.0, "act": 0.0}

        def pop_one():
            label, fn, cost = filler.popleft()
            fn()
            if label is not None:
                done.add(label)
            est["pe"] += cost

        def need(label):
            while label not in done:
                assert filler, f"gate {label} not in filler"
                pop_one()

        def budget_pops():
            while filler and est["pe"] + RESERVE < est["act"]:
                pop_one()

        def ph1_steps(j):
            """Phase-1 QKV projection for query block j as filler items."""
            jsl = slice(j * TQ, (j + 1) * TQ)
            xr_j = xr_cache[j]
            items = []

            def m_group(m):
                cell = {}

                def mk(d):
                    def fn():
                        if d == 0:
                            cell["ps"] = ps_sh.tile(
                                [128, TQ], F32, tag="sh", name=f"psqk{j}_{m}")
                        ps = cell["ps"]
                        last = (d == NDCH - 1) and not qkv_bias
                        nc.tensor.matmul(
                            ps[:], wqk_r[d][:, m * 128:(m + 1) * 128],
                            xr_j[d][:], start=(d == 0), stop=last)
                        if d == NDCH - 1:
                            if qkv_bias:
                                nc.tensor.matmul(
                                    ps[:], bqk_r[0:1, m * 128:(m + 1) * 128],
                                    ones512_r[:], start=False, stop=True)
                            dst = q2[m] if m < 4 else kT[m - 4]
                            nc.vector.tensor_copy(dst[:, jsl], ps[:])
                    return fn

                return ([(None, mk(d), EST_FILL) for d in range(NDCH - 1)]
                        + [(("m", j, m), mk(NDCH - 1), EST_FILL + 60)])

            def v_group(tt):
                c = tt % 4
                cell = {}

                def mk(d):
                    def fn():
                        if d == 0:
                            cell["ps"] = ps_sh.tile(
                                [128, DL], F32, tag="sh", name=f"psv{tt}")
                        ps = cell["ps"]
                        last = (d == NDCH - 1) and not qkv_bias
                        nc.tensor.matmul(
                            ps[:], xr_j[d][:, c * 128:(c + 1) * 128],
                            wv_r[d][:], start=(d == 0), stop=last)
                        if d == NDCH - 1:
                            if qkv_bias:
                                nc.tensor.matmul(
                                    ps[:], ones_r[:, 0:128], bv_r[:],
                                    start=False, stop=True)
                            src = ps.rearrange("p (h x) -> p h x", h=HL)
                            dst = vs[tt][:, 0:HL * 65].rearrange(
                                "p (h x) -> p h x", x=65)[:, :, 0:64]
                            nc.vector.tensor_copy(dst, src)
                    return fn

                return ([(None, mk(d), EST_FILL) for d in range(NDCH - 1)]
                        + [(("v", j, tt % 4), mk(NDCH - 1), EST_FILL + 60)])

            # order: what attention needs first -- pair-0 Q/K, then V, then
            # the remaining pairs' Q/K.
            items += m_group(0) + m_group(4)
            for tt in range(4 * j, 4 * j + 4):
                items += v_group(tt)
            for i in range(1, 4):
                items += m_group(i) + m_group(4 + i)
            return items

        def proj_steps(j):
            items = []
            for t in range(4 * j, 4 * j + 4):
                for nb in range(2):
                    def fn(t=t, nb=nb):
                        nsl = slice(nb * 512, (nb + 1) * 512)
                        ps3 = ps_sh.tile([128, TQ], F32, tag="sh",
                                         name=f"ps3_{t}_{nb}")
                        for k in range(4):
                            nc.tensor.matmul(
                                ps3[:], yT[k][:, t * 128:(t + 1) * 128],
                                wp_r[k][:, nsl], start=(k == 0), stop=(k == 3))
                        ot = opool.tile([128, TQ], F32, tag="ot",
                                        name=f"ot{t}_{nb}")
                        nc.vector.tensor_copy(ot[:], ps3[:])
                        nc.sync.dma_start(out_d[t * 128:(t + 1) * 128, nsl],
                                          ot[:])
                    items.append((None, fn, 4 * EST_FILL))
            return items

        norm_q = deque()   # deferred [pb matmul + yT mul] closures
        proj_reserve = []  # proj items held back to fill the last attn block

        def drain_norm():
            while norm_q:
                norm_q.popleft()()
                est["pe"] += 2 * EST_FILL

        # ---------------- main pipelined loop ----------------
        xload(0)
        if causal:
            xload(1)
            filler.extend(ph1_steps(0))
        else:
            for j in range(1, NQB):
                xload(j)
            for j in range(NQB):
                filler.extend(ph1_steps(j))

        for j in range(NQB):
            jsl = slice(j * TQ, (j + 1) * TQ)
            cs = list(range(4 * (j + 1))) if causal else list(range(NKC))
            if causal and j + 2 < NQB:
                xload(j + 2)
            if causal and j + 1 < NQB:
                filler.extend(ph1_steps(j + 1))
            if causal and j == NQB - 1:
                filler.extend(proj_reserve)
                proj_reserve.clear()

            for i in range(4):          # head pair (2i, 2i+1)
                hA, hB = 2 * i, 2 * i + 1
                need(("m", j, i))
                poA = ps_o.tile([128, TQ], F32, tag="po", name=f"poA{j}_{i}")
                poB = ps_o.tile([128, TQ], F32, tag="po", name=f"poB{j}_{i}")

                pend = deque()  # pipeline: PV(c) emitted after QK(c+2)
                first_pv = [True]

                def pv_pop(stop):
                    pc, ppt = pend.popleft()
                    need(("v", pc // 4, pc % 4))
                    pv_emit(pc, ppt, first_pv[0], stop)
                    first_pv[0] = False
                def qskip(c):
                    # fully-masked leading query columns of a diagonal chunk
                    if causal and c >= 4 * j:
                        return (c - 4 * j) * TKC
                    return 0

                def pv_emit(pc, ppt, start, stop):
                    k0 = qskip(pc)
                    nc.tensor.matmul(
                        poA[:, k0:TQ], vs[pc][:, hA * 65:hA * 65 + 128],
                        ppt[:, k0:TQ], start=start, stop=stop)
                    nc.tensor.matmul(
                        poB[:, k0:TQ], vs[pc][:, hB * 65:hB * 65 + 128],
                        ppt[:, TQ + k0:2 * TQ], start=start, stop=stop)
                    est["pe"] += EST_PV

                for ci, c in enumerate(cs):
                    need(("m", c // 4, 4 + i))
                    csl = slice(c * TKC, (c + 1) * TKC)
                    k0 = qskip(c)
                    ss = ps_s.tile([TKC, 2 * TQ], F32, tag="ss",
                                   name=f"ss{j}_{i}_{c}")
                    nc.tensor.matmul(ss[:, k0:TQ], kT[i][0:64, csl],
                                     q2[i][0:64, j * TQ + k0:(j + 1) * TQ],
                                     start=True, stop=True)
                    nc.tensor.matmul(ss[:, TQ + k0:2 * TQ], kT[i][64:128, csl],
                                     q2[i][64:128, j * TQ + k0:(j + 1) * TQ],
                                     start=True, stop=True)
                    est["pe"] += EST_QK
                    pt = ppool.tile([TKC, 2 * TQ], BF16, tag="pt",
                                    name=f"pt{j}_{i}_{c}")
                    nc.scalar.activation(pt[:], ss[:], EXP, scale=0.125)
                    est["act"] += EST_EXP
                    if causal and c >= 4 * j:
                        # only the 128-wide diagonal band is partially masked
                        bsl = slice(k0, k0 + TKC)
                        nc.vector.tensor_mul(pt[:, bsl], pt[:, bsl], maskb[:])
                        bslB = slice(TQ + k0, TQ + k0 + TKC)
                        nc.vector.tensor_mul(pt[:, bslB], pt[:, bslB],
                                             maskb[:])
                    if ci == 2:
                        drain_norm()   # previous pair's deferred norm
                    if len(pend) >= 2:
                        pv_pop(False)
                    pend.append((c, pt))
                    budget_pops()
                if len(cs) < 3:
                    drain_norm()
                while len(pend) > 1:
                    pv_pop(False)
                budget_pops()
                pv_pop(True)

                # immediate DVE part of softmax normalization
                rr, osb = [], []
                for h, po in ((hA, poA), (hB, poB)):
                    sums = npool.tile([1, TQ], F32, tag="sums",
                                      name=f"sm{j}_{h}")
                    nc.vector.tensor_copy(sums[:], po[64:65, :])
                    o_sb = npool.tile([64, TQ], BF16, tag="o_sb",
                                      name=f"ob{j}_{h}")
                    nc.vector.tensor_copy(o_sb[:], po[0:64, :])
                    recip = npool.tile([1, TQ], F32, tag="recip",
                                       name=f"rc{j}_{h}")
                    recip_r = npool.tile([1, TQ], F32R, tag="recip_r",
                                         name=f"rr{j}_{h}")
                    nc.vector.reciprocal_approx_fast(
                        out=recip[:], in_=sums[:])
                    nc.vector.tensor_copy(recip_r[:], recip[:])
                    rr.append(recip_r)
                    osb.append(o_sb)

                def norm_fn(i=i, jsl=jsl, osb=osb, rr=rr, j=j, hA=hA):
                    for hp in (0, 1):
                        pb = ps_sh.tile([64, TQ], F32, tag="sh",
                                        name=f"pb{j}_{hA + hp}")
                        nc.tensor.matmul(pb[:], ones_r[:, 0:64], rr[hp][:],
                                         start=True, stop=True)
                        nc.vector.tensor_mul(
                            yT[i][hp * 64:(hp + 1) * 64, jsl],
                            osb[hp][:], pb[:])
                norm_q.append(norm_fn)

            drain_norm()               # before proj(j) can be emitted
            items = proj_steps(j)
            if causal and j < 2:       # hold some proj back to fill attn(3)
                filler.extend(items[:4])
                proj_reserve.extend(items[4:])
            else:
                filler.extend(items)

        while filler:                  # flush remaining projection work
            pop_one()

    nc.compile()
    return nc


def _get_nc(causal: bool, qkv_bias: bool = False):
    key = (causal, qkv_bias)
    if key not in _CACHE:
        _CACHE[key] = _build(causal, qkv_bias)
    return _CACHE[key]


def _host_masks() -> np.ndarray:
    i = np.arange(TKC)[:, None]
    jj = np.arange(TKC)[None, :]
    return np.ascontiguousarray(
        (jj >= i).astype(np.float32).astype(ml_dtypes.bfloat16))


def _make_in_maps(x, W_qkv, b_qkv, W_proj):
    masks_np = _host_masks()
    in_maps = []
    for core in range(N_CORES):
        b, g = core // 2, core % 2
        qc = slice(g * DL, (g + 1) * DL)
        kc = slice(D + g * DL, D + (g + 1) * DL)
        vc = slice(2 * D + g * DL, 2 * D + (g + 1) * DL)
        in_maps.append({
            "xT": np.ascontiguousarray(x[b].T),
            "wqk": np.ascontiguousarray(
                np.concatenate([W_qkv[:, qc], W_qkv[:, kc]], axis=1)),
            "wv": np.ascontiguousarray(W_qkv[:, vc]),
            "bqk": np.ascontiguousarray(
                np.concatenate([b_qkv[qc], b_qkv[kc]]).reshape(1, 2 * DL)),
            "bv": np.ascontiguousarray(b_qkv[vc].reshape(1, DL)),
            "wproj": np.ascontiguousarray(W_proj[g * DL:(g + 1) * DL, :]),
            "masks": masks_np,
        })
    return in_maps


def kernel(x, mask, W_qkv, b_qkv, W_proj, b_proj):
    x = np.asarray(x, dtype=np.float32)
    mask2d = np.asarray(mask, dtype=np.int32).reshape(T, T)
    W_qkv = np.asarray(W_qkv, dtype=np.float32)
    b_qkv = np.asarray(b_qkv, dtype=np.float32)
    W_proj = np.asarray(W_proj, dtype=np.float32)
    b_proj = np.asarray(b_proj, dtype=np.float32)

    if np.array_equal(mask2d, np.tril(np.ones((T, T), dtype=np.int32))):
        causal = True
    elif np.all(mask2d == 1):
        causal = False
    else:
        raise NotImplementedError("only causal (tril) or all-ones masks")

    qkv_bias = bool(np.any(b_qkv != 0.0))
    nc = _get_nc(causal, qkv_bias)
    in_maps = _make_in_maps(x, W_qkv, b_qkv, W_proj)
    res = run_bass_kernel_spmd(nc, in_maps, core_ids=list(range(N_CORES)))
    out = np.empty((B, T, D), dtype=np.float32)
    for b in range(B):
        out[b] = (res.results[2 * b]["out"] + res.results[2 * b + 1]["out"]
                  + b_proj[None, :])
    return out


# revision 9
# speedup vs baseline: 1.4489x; 1.0938x over previous
"""Multi-head causal self-attention for TRN2, 8 NeuronCores.

Sharding: core i handles (batch b = i//2, head-group g = i%2); each head-group
is 8 of the 16 heads.  Per core everything is computed in "transposed" space so
no on-device transposes are needed.

v2 vs baseline:
  * QK attention matmuls are row-tiled: the two heads of a pair run as
    concurrent K=64 matmuls on PE row-groups (0,0)/(64,0) writing the two
    halves (separate PSUM banks) of one [128, 1024] score tile, so both
    heads' S^T cost one 512-col stream instead of two.
  * Single software-pipelined loop: phase-1 QKV projection work for block
    j+1 and the output projection for block j-1 are emitted as fine-grained
    "filler" matmuls inside the ACT(exp)-paced attention chunk loop, with
    FIFO gating so attention never waits on un-emitted producers.
  * No ACT in phase 1: QKV biases (when nonzero) are rank-1 matmuls; PSUM
    drains are DVE copies.  ACT does only the 160 softmax exp calls.
  * x^T f32->bf16 casts run on the otherwise-idle GPSIMD engine.
  * softmax normalization: reciprocal on DVE, [1,q]->[64,q] broadcast via a
    K=1 matmul into the shared PSUM pool, deferred past the next pair's
    first chunk so the PE never stalls on the DVE reciprocal chain.
"""

import numpy as np
import ml_dtypes
from collections import deque
from contextlib import ExitStack

import concourse.bass as bass
import concourse.mybir as mybir
import concourse.tile as tile
from concourse import bacc
from concourse.bass_utils import run_bass_kernel_spmd

B, T, D, H = 4, 2048, 1024, 16
DK = 64            # head dim
HL = 8             # heads per core
DL = HL * DK       # 512 local head dims per core
N_CORES = 8

F32 = mybir.dt.float32
F32R = mybir.dt.float32r
BF16 = mybir.dt.bfloat16
EXP = mybir.ActivationFunctionType.Exp

TQ = 512           # tq block size
TKC = 128          # tk chunk size
NQB = T // TQ      # 4
NKC = T // TKC     # 16
NDCH = D // 128    # 8 contraction chunks over D
VSW = HL * 65 + 64  # staged-V width: 8*[V_h|1] + ones tail pad for M=128 lhsT

# rough per-instruction engine-busy estimates (ns) for the static scheduler
EST_QK = 350       # two concurrent row-tiled K=64 matmuls, N=512
EST_PV = 560       # two K=128 matmuls, N=512
EST_EXP = 1080     # ACT exp on [128, 1024] (measured)
EST_FILL = 300     # one N=512 matmul
RESERVE = 800

_CACHE = {}


def _build(causal: bool, qkv_bias: bool):
    nc = bacc.Bacc("TRN2", target_bir_lowering=False, debug=False,
                   num_devices=N_CORES)
    xT_d = nc.dram_tensor("xT", [D, T], BF16, kind="ExternalInput").ap()
    wqk_d = nc.dram_tensor("wqk", [D, 2 * DL], BF16, kind="ExternalInput").ap()
    wv_d = nc.dram_tensor("wv", [D, DL], BF16, kind="ExternalInput").ap()
    wp_d = nc.dram_tensor("wproj", [DL, D], BF16, kind="ExternalInput").ap()
    bqk_d = nc.dram_tensor("bqk", [1, 2 * DL], F32, kind="ExternalInput").ap()
    bv_d = nc.dram_tensor("bv", [1, DL], F32, kind="ExternalInput").ap()
    masks_d = nc.dram_tensor("masks", [TKC, TKC], BF16,
                             kind="ExternalInput").ap()
    out_d = nc.dram_tensor("out", [T, D], F32, kind="ExternalOutput").ap()

    with tile.TileContext(nc) as tc, ExitStack() as top:
        persist = top.enter_context(tc.tile_pool(name="persist", bufs=1))
        xrpool = top.enter_context(
            tc.tile_pool(name="xrpool", bufs=2 if causal else 4))
        ps_s = top.enter_context(tc.tile_pool(name="ps_s", bufs=2, space="PSUM"))
        ps_o = top.enter_context(tc.tile_pool(name="ps_o", bufs=2, space="PSUM"))
        ps_sh = top.enter_context(tc.tile_pool(name="ps_sh", bufs=2, space="PSUM"))
        ppool = top.enter_context(tc.tile_pool(name="ppool", bufs=8))
        npool = top.enter_context(tc.tile_pool(name="npool", bufs=2))
        opool = top.enter_context(tc.tile_pool(name="opool", bufs=3))

        # ---------------- persistent tiles ----------------
        q2 = [persist.tile([128, T], BF16, tag=f"q2{i}", name=f"q2{i}")
              for i in range(4)]       # head-pair packed Q^T
        kT = [persist.tile([128, T], BF16, tag=f"kT{i}", name=f"kT{i}")
              for i in range(4)]       # head-pair packed K^T
        vs = [persist.tile([128, VSW], BF16, tag=f"vs{t}", name=f"vs{t}")
              for t in range(NKC)]     # staged V: [V_h|1]*8 + ones tail
        yT = [persist.tile([128, T], BF16, tag=f"yT{i}", name=f"yT{i}")
              for i in range(4)]
        wqk_r = [persist.tile([128, 2 * DL], BF16, tag=f"wqk{d}", name=f"wqk{d}")
                 for d in range(NDCH)]
        wv_r = [persist.tile([128, DL], BF16, tag=f"wv{d}", name=f"wv{d}")
                for d in range(NDCH)]
        wp_r = [persist.tile([128, D], BF16, tag=f"wp{k}", name=f"wp{k}")
                for k in range(4)]
        ones_r = persist.tile([1, 128], F32R, tag="ones_r", name="ones_r")
        maskb = None
        if causal:
            maskb = persist.tile([TKC, TKC], BF16, tag="maskb", name="maskb")
            nc.gpsimd.dma_start(maskb[:], masks_d)

        # ---------------- preamble ----------------
        initp = top.enter_context(tc.tile_pool(name="initp", bufs=1))
        ones_f = initp.tile([1, 512], F32, tag="ones_f", name="ones_f")
        nc.vector.memset(ones_f[:], 1.0)
        nc.vector.tensor_copy(ones_r[:], ones_f[:, 0:128])
        bqk_r = bv_r = ones512_r = None
        if qkv_bias:
            ones512_r = initp.tile([1, 512], F32R, tag="ones512_r",
                                   name="ones512_r")
            nc.vector.tensor_copy(ones512_r[:], ones_f[:])
            bqk_f = initp.tile([1, 2 * DL], F32, tag="bqk_f", name="bqk_f")
            nc.gpsimd.dma_start(bqk_f[:], bqk_d)
            bqk_r = initp.tile([1, 2 * DL], F32R, tag="bqk_r", name="bqk_r")
            nc.vector.tensor_copy(bqk_r[:], bqk_f[:])
            bv_f = initp.tile([1, DL], F32, tag="bv_f", name="bv_f")
            nc.gpsimd.dma_start(bv_f[:], bv_d)
            bv_r = initp.tile([1, DL], F32R, tag="bv_r", name="bv_r")
            nc.vector.tensor_copy(bv_r[:], bv_f[:])

        # weights: DMA stage f32 -> DVE cast to bf16 resident copies
        dmaq = [nc.gpsimd, nc.scalar]
        for d in range(NDCH):
            dmaq[d % 2].dma_start(wqk_r[d][:], wqk_d[d * 128:(d + 1) * 128, :])
        for d in range(NDCH):
            dmaq[d % 2].dma_start(wv_r[d][:], wv_d[d * 128:(d + 1) * 128, :])
        for k in range(4):
            dmaq[k % 2].dma_start(wp_r[k][:], wp_d[k * 128:(k + 1) * 128, :])

        # staged-V tiles start as all-ones; the V copies overwrite the V
        # columns and leave the |1 columns and the tail as ones.
        for t in range(NKC):
            nc.vector.memset(vs[t][:], 1.0)

        # ---------------- x loads (DMA + gpsimd cast) ----------------
        xr_cache = {}

        def xload(j):
            jsl = slice(j * TQ, (j + 1) * TQ)
            xr_j = []
            for d in range(NDCH):
                xr_t = xrpool.tile([128, TQ], BF16, tag=f"xr{d}",
                                   name=f"xr{j}_{d}")
                nc.sync.dma_start(xr_t[:], xT_d[d * 128:(d + 1) * 128, jsl])
                xr_j.append(xr_t)
            xr_cache[j] = xr_j

        # ---------------- filler machinery ----------------
        filler = deque()   # items: (label_or_None, fn, est_pe_ns)
        done = set()
        est = {"pe": 0.0, "act": 0.0}

        def pop_one():
            label, fn, cost = filler.popleft()
            fn()
            if label is not None:
                done.add(label)
            est["pe"] += cost

        def need(label):
            while label not in done:
                assert filler, f"gate {label} not in filler"
                pop_one()

        def budget_pops():
            while filler and est["pe"] + RESERVE < est["act"]:
                pop_one()

        def ph1_steps(j):
            """Phase-1 QKV projection for query block j as filler items."""
            jsl = slice(j * TQ, (j + 1) * TQ)
            xr_j = xr_cache[j]
            items = []

            def m_group(m):
                cell = {}

                def mk(d):
                    def fn():
                        if d == 0:
                            cell["ps"] = ps_sh.tile(
                                [128, TQ], F32, tag="sh", name=f"psqk{j}_{m}")
                        ps = cell["ps"]
                        last = (d == NDCH - 1) and not qkv_bias
                        nc.tensor.matmul(
                            ps[:], wqk_r[d][:, m * 128:(m + 1) * 128],
                            xr_j[d][:], start=(d == 0), stop=last)
                        if d == NDCH - 1:
                            if qkv_bias:
                                nc.tensor.matmul(
                                    ps[:], bqk_r[0:1, m * 128:(m + 1) * 128],
                                    ones512_r[:], start=False, stop=True)
                            dst = q2[m] if m < 4 else kT[m - 4]
                            nc.vector.tensor_copy(dst[:, jsl], ps[:])
                    return fn

                return ([(None, mk(d), EST_FILL) for d in range(NDCH - 1)]
                        + [(("m", j, m), mk(NDCH - 1), EST_FILL + 60)])

            def v_group(tt):
                c = tt % 4
                cell = {}

                def mk(d):
                    def fn():
                        if d == 0:
                            cell["ps"] = ps_sh.tile(
                                [128, DL], F32, tag="sh", name=f"psv{tt}")
                        ps = cell["ps"]
                        last = (d == NDCH - 1) and not qkv_bias
                        nc.tensor.matmul(
                            ps[:], xr_j[d][:, c * 128:(c + 1) * 128],
                            wv_r[d][:], start=(d == 0), stop=last)
                        if d == NDCH - 1:
                            if qkv_bias:
                                nc.tensor.matmul(
                                    ps[:], ones_r[:, 0:128], bv_r[:],
                                    start=False, stop=True)
                            src = ps.rearrange("p (h x) -> p h x", h=HL)
                            dst = vs[tt][:, 0:HL * 65].rearrange(
                                "p (h x) -> p h x", x=65)[:, :, 0:64]
                            nc.vector.tensor_copy(dst, src)
                    return fn

                return ([(None, mk(d), EST_FILL) for d in range(NDCH - 1)]
                        + [(("v", j, tt % 4), mk(NDCH - 1), EST_FILL + 60)])

            # order: what attention needs first -- pair-0 Q/K, then V, then
            # the remaining pairs' Q/K.
            items += m_group(0) + m_group(4)
            for tt in range(4 * j, 4 * j + 4):
                items += v_group(tt)
            for i in range(1, 4):
                items += m_group(i) + m_group(4 + i)
            return items

        def proj_steps(j):
            items = []
            for t in range(4 * j, 4 * j + 4):
                for nb in range(2):
                    def fn(t=t, nb=nb):
                        nsl = slice(nb * 512, (nb + 1) * 512)
                        ps3 = ps_sh.tile([128, TQ], F32, tag="sh",
                                         name=f"ps3_{t}_{nb}")
                        for k in range(4):
                            nc.tensor.matmul(
                                ps3[:], yT[k][:, t * 128:(t + 1) * 128],
                                wp_r[k][:, nsl], start=(k == 0), stop=(k == 3))
                        ot = opool.tile([128, TQ], F32, tag="ot",
                                        name=f"ot{t}_{nb}")
                        nc.vector.tensor_copy(ot[:], ps3[:])
                        nc.sync.dma_start(out_d[t * 128:(t + 1) * 128, nsl],
                                          ot[:])
                    items.append((None, fn, 4 * EST_FILL))
            return items

        norm_q = deque()   # deferred [pb matmul + yT mul] closures
        proj_reserve = []  # proj items held back to fill the last attn block

        def drain_norm():
            while norm_q:
                norm_q.popleft()()
                est["pe"] += 2 * EST_FILL

        # ---------------- main pipelined loop ----------------
        xload(0)
        if causal:
            xload(1)
            filler.extend(ph1_steps(0))
        else:
            for j in range(1, NQB):
                xload(j)
            for j in range(NQB):
                filler.extend(ph1_steps(j))

        for j in range(NQB):
            jsl = slice(j * TQ, (j + 1) * TQ)
            cs = list(range(4 * (j + 1))) if causal else list(range(NKC))
            if causal and j + 2 < NQB:
                xload(j + 2)
            if causal and j + 1 < NQB:
                filler.extend(ph1_steps(j + 1))
            if causal and j == NQB - 1:
                filler.extend(proj_reserve)
                proj_reserve.clear()

            for i in range(4):          # head pair (2i, 2i+1)
                hA, hB = 2 * i, 2 * i + 1
                need(("m", j, i))
                poA = ps_o.tile([128, TQ], F32, tag="po", name=f"poA{j}_{i}")
                poB = ps_o.tile([128, TQ], F32, tag="po", name=f"poB{j}_{i}")

                pend = deque()  # pipeline: PV(c) emitted after QK(c+2)
                first_pv = [True]

                def pv_pop(stop):
                    pc, ppt = pend.popleft()
                    need(("v", pc // 4, pc % 4))
                    pv_emit(pc, ppt, first_pv[0], stop)
                    first_pv[0] = False
                def qskip(c):
                    # fully-masked leading query columns of a diagonal chunk
                    if causal and c >= 4 * j:
                        return (c - 4 * j) * TKC
                    return 0

                def pv_emit(pc, ppt, start, stop):
                    k0 = qskip(pc)
                    nc.tensor.matmul(
                        poA[:, k0:TQ], vs[pc][:, hA * 65:hA * 65 + 128],
                        ppt[:, k0:TQ], start=start, stop=stop)
                    nc.tensor.matmul(
                        poB[:, k0:TQ], vs[pc][:, hB * 65:hB * 65 + 128],
                        ppt[:, TQ + k0:2 * TQ], start=start, stop=stop)
                    est["pe"] += EST_PV

                for ci, c in enumerate(cs):
                    need(("m", c // 4, 4 + i))
                    csl = slice(c * TKC, (c + 1) * TKC)
                    k0 = qskip(c)
                    ss = ps_s.tile([TKC, 2 * TQ], F32, tag="ss",
                                   name=f"ss{j}_{i}_{c}")
                    nc.tensor.matmul(ss[:, k0:TQ], kT[i][0:64, csl],
                                     q2[i][0:64, j * TQ + k0:(j + 1) * TQ],
                                     start=True, stop=True)
                    nc.tensor.matmul(ss[:, TQ + k0:2 * TQ], kT[i][64:128, csl],
                                     q2[i][64:128, j * TQ + k0:(j + 1) * TQ],
                                     start=True, stop=True)
                    est["pe"] += EST_QK
                    pt = ppool.tile([TKC, 2 * TQ], BF16, tag="pt",
                                    name=f"pt{j}_{i}_{c}")
                    nc.scalar.activation(pt[:], ss[:], EXP, scale=0.125)
                    est["act"] += EST_EXP
                    if causal and c >= 4 * j:
                        # only the 128-wide diagonal band is partially masked
                        bsl = slice(k0, k0 + TKC)
                        nc.vector.tensor_mul(pt[:, bsl], pt[:, bsl], maskb[:])
                        bslB = slice(TQ + k0, TQ + k0 + TKC)
                        nc.vector.tensor_mul(pt[:, bslB], pt[:, bslB],
                                             maskb[:])
                    if ci == 2:
                        drain_norm()   # previous pair's deferred norm
                    if len(pend) >= 2:
                        pv_pop(False)
                    pend.append((c, pt))
                    budget_pops()
                if len(cs) < 3:
                    drain_norm()
                while len(pend) > 1:
                    pv_pop(False)
                budget_pops()
                pv_pop(True)

                # immediate DVE part of softmax normalization
                rr, osb = [], []
                for h, po in ((hA, poA), (hB, poB)):
                    sums = npool.tile([1, TQ], F32, tag="sums",
                                      name=f"sm{j}_{h}")
                    nc.vector.tensor_copy(sums[:], po[64:65, :])
                    o_sb = npool.tile([64, TQ], BF16, tag="o_sb",
                                      name=f"ob{j}_{h}")
                    nc.vector.tensor_copy(o_sb[:], po[0:64, :])
                    recip = npool.tile([1, TQ], F32, tag="recip",
                                       name=f"rc{j}_{h}")
                    recip_r = npool.tile([1, TQ], F32R, tag="recip_r",
                                         name=f"rr{j}_{h}")
                    nc.vector.reciprocal_approx_fast(
                        out=recip[:], in_=sums[:])
                    nc.vector.tensor_copy(recip_r[:], recip[:])
                    rr.append(recip_r)
                    osb.append(o_sb)

                def norm_fn(i=i, jsl=jsl, osb=osb, rr=rr, j=j, hA=hA):
                    for hp in (0, 1):
                        pb = ps_sh.tile([64, TQ], F32, tag="sh",
                                        name=f"pb{j}_{hA + hp}")
                        nc.tensor.matmul(pb[:], ones_r[:, 0:64], rr[hp][:],
                                         start=True, stop=True)
                        nc.vector.tensor_mul(
                            yT[i][hp * 64:(hp + 1) * 64, jsl],
                            osb[hp][:], pb[:])
                norm_q.append(norm_fn)

            drain_norm()               # before proj(j) can be emitted
            items = proj_steps(j)
            if causal and j < 2:       # hold some proj back to fill attn(3)
                filler.extend(items[:4])
                proj_reserve.extend(items[4:])
            else:
                filler.extend(items)

        while filler:                  # flush remaining projection work
            pop_one()

    nc.compile()
    return nc


def _get_nc(causal: bool, qkv_bias: bool = False):
    key = (causal, qkv_bias)
    if key not in _CACHE:
        _CACHE[key] = _build(causal, qkv_bias)
    return _CACHE[key]


def _host_masks() -> np.ndarray:
    i = np.arange(TKC)[:, None]
    jj = np.arange(TKC)[None, :]
    return np.ascontiguousarray(
        (jj >= i).astype(np.float32).astype(ml_dtypes.bfloat16))


def _make_in_maps(x, W_qkv, b_qkv, W_proj):
    masks_np = _host_masks()
    in_maps = []
    for core in range(N_CORES):
        b, g = core // 2, core % 2
        qc = slice(g * DL, (g + 1) * DL)
        kc = slice(D + g * DL, D + (g + 1) * DL)
        vc = slice(2 * D + g * DL, 2 * D + (g + 1) * DL)
        bf = ml_dtypes.bfloat16
        in_maps.append({
            "xT": np.ascontiguousarray(x[b].T.astype(bf)),
            "wqk": np.ascontiguousarray(
                np.concatenate([W_qkv[:, qc], W_qkv[:, kc]],
                               axis=1).astype(bf)),
            "wv": np.ascontiguousarray(W_qkv[:, vc].astype(bf)),
            "bqk": np.ascontiguousarray(
                np.concatenate([b_qkv[qc], b_qkv[kc]]).reshape(1, 2 * DL)),
            "bv": np.ascontiguousarray(b_qkv[vc].reshape(1, DL)),
            "wproj": np.ascontiguousarray(
                W_proj[g * DL:(g + 1) * DL, :].astype(bf)),
            "masks": masks_np,
        })
    return in_maps


def kernel(x, mask, W_qkv, b_qkv, W_proj, b_proj):
    x = np.asarray(x, dtype=np.float32)
    mask2d = np.asarray(mask, dtype=np.int32).reshape(T, T)
    W_qkv = np.asarray(W_qkv, dtype=np.float32)
    b_qkv = np.asarray(b_qkv, dtype=np.float32)
    W_proj = np.asarray(W_proj, dtype=np.float32)
    b_proj = np.asarray(b_proj, dtype=np.float32)

    if np.array_equal(mask2d, np.tril(np.ones((T, T), dtype=np.int32))):
        causal = True
    elif np.all(mask2d == 1):
        causal = False
    else:
        raise NotImplementedError("only causal (tril) or all-ones masks")

    qkv_bias = bool(np.any(b_qkv != 0.0))
    nc = _get_nc(causal, qkv_bias)
    in_maps = _make_in_maps(x, W_qkv, b_qkv, W_proj)
    res = run_bass_kernel_spmd(nc, in_maps, core_ids=list(range(N_CORES)))
    out = np.empty((B, T, D), dtype=np.float32)
    for b in range(B):
        out[b] = (res.results[2 * b]["out"] + res.results[2 * b + 1]["out"]
                  + b_proj[None, :])
    return out


# revision 10
# speedup vs baseline: 1.4706x; 1.0150x over previous
"""Multi-head causal self-attention for TRN2, 8 NeuronCores.

Sharding: core i handles (batch b = i//2, head-group g = i%2); each head-group
is 8 of the 16 heads.  Per core everything is computed in "transposed" space so
no on-device transposes are needed.

v2 vs baseline:
  * QK attention matmuls are row-tiled: the two heads of a pair run as
    concurrent K=64 matmuls on PE row-groups (0,0)/(64,0) writing the two
    halves (separate PSUM banks) of one [128, 1024] score tile, so both
    heads' S^T cost one 512-col stream instead of two.
  * Single software-pipelined loop: phase-1 QKV projection work for block
    j+1 and the output projection for block j-1 are emitted as fine-grained
    "filler" matmuls inside the ACT(exp)-paced attention chunk loop, with
    FIFO gating so attention never waits on un-emitted producers.
  * No ACT in phase 1: QKV biases (when nonzero) are rank-1 matmuls; PSUM
    drains are DVE copies.  ACT does only the 160 softmax exp calls.
  * x^T f32->bf16 casts run on the otherwise-idle GPSIMD engine.
  * softmax normalization: reciprocal on DVE, [1,q]->[64,q] broadcast via a
    K=1 matmul into the shared PSUM pool, deferred past the next pair's
    first chunk so the PE never stalls on the DVE reciprocal chain.
"""

import numpy as np
import ml_dtypes
from collections import deque
from contextlib import ExitStack

import concourse.bass as bass
import concourse.mybir as mybir
import concourse.tile as tile
from concourse import bacc
from concourse.bass_utils import run_bass_kernel_spmd

B, T, D, H = 4, 2048, 1024, 16
DK = 64            # head dim
HL = 8             # heads per core
DL = HL * DK       # 512 local head dims per core
N_CORES = 8

F32 = mybir.dt.float32
F32R = mybir.dt.float32r
BF16 = mybir.dt.bfloat16
EXP = mybir.ActivationFunctionType.Exp

TQ = 512           # tq block size
TKC = 128          # tk chunk size
NQB = T // TQ      # 4
NKC = T // TKC     # 16
NDCH = D // 128    # 8 contraction chunks over D
VSW = HL * 65 + 64  # staged-V width: 8*[V_h|1] + ones tail pad for M=128 lhsT

# rough per-instruction engine-busy estimates (ns) for the static scheduler
EST_QK = 350       # two concurrent row-tiled K=64 matmuls, N=512
EST_PV = 560       # two K=128 matmuls, N=512
EST_EXP = 1010     # ACT exp on [128, 1024] (measured)
EST_FILL = 300     # one N=512 matmul
RESERVE = 800

_CACHE = {}


def _build(causal: bool, qkv_bias: bool):
    nc = bacc.Bacc("TRN2", target_bir_lowering=False, debug=False,
                   num_devices=N_CORES)
    xT_d = nc.dram_tensor("xT", [D, T], BF16, kind="ExternalInput").ap()
    wqk_d = nc.dram_tensor("wqk", [D, 2 * DL], BF16, kind="ExternalInput").ap()
    wv_d = nc.dram_tensor("wv", [D, DL], BF16, kind="ExternalInput").ap()
    wp_d = nc.dram_tensor("wproj", [DL, D], BF16, kind="ExternalInput").ap()
    bqk_d = nc.dram_tensor("bqk", [1, 2 * DL], F32, kind="ExternalInput").ap()
    bv_d = nc.dram_tensor("bv", [1, DL], F32, kind="ExternalInput").ap()
    masks_d = nc.dram_tensor("masks", [TKC, TKC], BF16,
                             kind="ExternalInput").ap()
    out_d = nc.dram_tensor("out", [T, D], F32, kind="ExternalOutput").ap()

    with tile.TileContext(nc) as tc, ExitStack() as top:
        persist = top.enter_context(tc.tile_pool(name="persist", bufs=1))
        xrpool = top.enter_context(
            tc.tile_pool(name="xrpool", bufs=2 if causal else 4))
        ps_s = top.enter_context(tc.tile_pool(name="ps_s", bufs=2, space="PSUM"))
        ps_o = top.enter_context(tc.tile_pool(name="ps_o", bufs=2, space="PSUM"))
        ps_sh = top.enter_context(tc.tile_pool(name="ps_sh", bufs=2, space="PSUM"))
        ppool = top.enter_context(tc.tile_pool(name="ppool", bufs=8))
        npool = top.enter_context(tc.tile_pool(name="npool", bufs=2))
        opool = top.enter_context(tc.tile_pool(name="opool", bufs=3))

        # ---------------- persistent tiles ----------------
        q2 = [persist.tile([128, T], BF16, tag=f"q2{i}", name=f"q2{i}")
              for i in range(4)]       # head-pair packed Q^T
        kT = [persist.tile([128, T], BF16, tag=f"kT{i}", name=f"kT{i}")
              for i in range(4)]       # head-pair packed K^T
        vs = [persist.tile([128, VSW], BF16, tag=f"vs{t}", name=f"vs{t}")
              for t in range(NKC)]     # staged V: [V_h|1]*8 + ones tail
        yT = [persist.tile([128, T], BF16, tag=f"yT{i}", name=f"yT{i}")
              for i in range(4)]
        wqk_r = [persist.tile([128, 2 * DL], BF16, tag=f"wqk{d}", name=f"wqk{d}")
                 for d in range(NDCH)]
        wv_r = [persist.tile([128, DL], BF16, tag=f"wv{d}", name=f"wv{d}")
                for d in range(NDCH)]
        wp_r = [persist.tile([128, D], BF16, tag=f"wp{k}", name=f"wp{k}")
                for k in range(4)]
        ones_r = persist.tile([1, 128], F32R, tag="ones_r", name="ones_r")
        maskb = None
        if causal:
            maskb = persist.tile([TKC, TKC], BF16, tag="maskb", name="maskb")
            nc.gpsimd.dma_start(maskb[:], masks_d)

        # ---------------- preamble ----------------
        initp = top.enter_context(tc.tile_pool(name="initp", bufs=1))
        ones_f = initp.tile([1, 512], F32, tag="ones_f", name="ones_f")
        nc.vector.memset(ones_f[:], 1.0)
        nc.vector.tensor_copy(ones_r[:], ones_f[:, 0:128])
        bqk_r = bv_r = ones512_r = None
        if qkv_bias:
            ones512_r = initp.tile([1, 512], F32R, tag="ones512_r",
                                   name="ones512_r")
            nc.vector.tensor_copy(ones512_r[:], ones_f[:])
            bqk_f = initp.tile([1, 2 * DL], F32, tag="bqk_f", name="bqk_f")
            nc.gpsimd.dma_start(bqk_f[:], bqk_d)
            bqk_r = initp.tile([1, 2 * DL], F32R, tag="bqk_r", name="bqk_r")
            nc.vector.tensor_copy(bqk_r[:], bqk_f[:])
            bv_f = initp.tile([1, DL], F32, tag="bv_f", name="bv_f")
            nc.gpsimd.dma_start(bv_f[:], bv_d)
            bv_r = initp.tile([1, DL], F32R, tag="bv_r", name="bv_r")
            nc.vector.tensor_copy(bv_r[:], bv_f[:])

        # weights: DMA stage f32 -> DVE cast to bf16 resident copies
        dmaq = [nc.gpsimd, nc.scalar]
        for d in range(NDCH):
            dmaq[d % 2].dma_start(wqk_r[d][:], wqk_d[d * 128:(d + 1) * 128, :])
        for d in range(NDCH):
            dmaq[d % 2].dma_start(wv_r[d][:], wv_d[d * 128:(d + 1) * 128, :])
        for k in range(4):
            dmaq[k % 2].dma_start(wp_r[k][:], wp_d[k * 128:(k + 1) * 128, :])

        # staged-V tiles start as all-ones; the V copies overwrite the V
        # columns and leave the |1 columns and the tail as ones.
        for t in range(NKC):
            nc.vector.memset(vs[t][:], 1.0)

        # ---------------- x loads (DMA + gpsimd cast) ----------------
        xr_cache = {}

        def xload(j):
            jsl = slice(j * TQ, (j + 1) * TQ)
            xr_j = []
            for d in range(NDCH):
                xr_t = xrpool.tile([128, TQ], BF16, tag=f"xr{d}",
                                   name=f"xr{j}_{d}")
                nc.sync.dma_start(xr_t[:], xT_d[d * 128:(d + 1) * 128, jsl])
                xr_j.append(xr_t)
            xr_cache[j] = xr_j

        # ---------------- filler machinery ----------------
        filler = deque()   # items: (label_or_None, fn, est_pe_ns)
        done = set()
        est = {"pe": 0.0, "act": 0.0}

        def pop_one():
            label, fn, cost = filler.popleft()
            fn()
            if label is not None:
                done.add(label)
            est["pe"] += cost

        def need(label):
            while label not in done:
                assert filler, f"gate {label} not in filler"
                pop_one()

        def budget_pops():
            while filler and est["pe"] + RESERVE < est["act"]:
                pop_one()

        def ph1_steps(j):
            """Phase-1 QKV projection for query block j as filler items."""
            jsl = slice(j * TQ, (j + 1) * TQ)
            xr_j = xr_cache[j]
            items = []

            def m_group(m):
                cell = {}

                def mk(d):
                    def fn():
                        if d == 0:
                            cell["ps"] = ps_sh.tile(
                                [128, TQ], F32, tag="sh", name=f"psqk{j}_{m}")
                        ps = cell["ps"]
                        last = (d == NDCH - 1) and not qkv_bias
                        nc.tensor.matmul(
                            ps[:], wqk_r[d][:, m * 128:(m + 1) * 128],
                            xr_j[d][:], start=(d == 0), stop=last)
                        if d == NDCH - 1:
                            if qkv_bias:
                                nc.tensor.matmul(
                                    ps[:], bqk_r[0:1, m * 128:(m + 1) * 128],
                                    ones512_r[:], start=False, stop=True)
                            dst = q2[m] if m < 4 else kT[m - 4]
                            nc.vector.tensor_copy(dst[:, jsl], ps[:])
                    return fn

                return ([(None, mk(d), EST_FILL) for d in range(NDCH - 1)]
                        + [(("m", j, m), mk(NDCH - 1), EST_FILL + 60)])

            def v_group(tt):
                c = tt % 4
                cell = {}

                def mk(d):
                    def fn():
                        if d == 0:
                            cell["ps"] = ps_sh.tile(
                                [128, DL], F32, tag="sh", name=f"psv{tt}")
                        ps = cell["ps"]
                        last = (d == NDCH - 1) and not qkv_bias
                        nc.tensor.matmul(
                            ps[:], xr_j[d][:, c * 128:(c + 1) * 128],
                            wv_r[d][:], start=(d == 0), stop=last)
                        if d == NDCH - 1:
                            if qkv_bias:
                                nc.tensor.matmul(
                                    ps[:], ones_r[:, 0:128], bv_r[:],
                                    start=False, stop=True)
                            src = ps.rearrange("p (h x) -> p h x", h=HL)
                            dst = vs[tt][:, 0:HL * 65].rearrange(
                                "p (h x) -> p h x", x=65)[:, :, 0:64]
                            nc.vector.tensor_copy(dst, src)
                    return fn

                return ([(None, mk(d), EST_FILL) for d in range(NDCH - 1)]
                        + [(("v", j, tt % 4), mk(NDCH - 1), EST_FILL + 60)])

            # order: what attention needs first -- pair-0 Q/K, then V, then
            # the remaining pairs' Q/K.
            items += m_group(0) + m_group(4)
            for tt in range(4 * j, 4 * j + 4):
                items += v_group(tt)
            for i in range(1, 4):
                items += m_group(i) + m_group(4 + i)
            return items

        def proj_steps(j):
            items = []
            for t in range(4 * j, 4 * j + 4):
                for nb in range(2):
                    def fn(t=t, nb=nb):
                        nsl = slice(nb * 512, (nb + 1) * 512)
                        ps3 = ps_sh.tile([128, TQ], F32, tag="sh",
                                         name=f"ps3_{t}_{nb}")
                        for k in range(4):
                            nc.tensor.matmul(
                                ps3[:], yT[k][:, t * 128:(t + 1) * 128],
                                wp_r[k][:, nsl], start=(k == 0), stop=(k == 3))
                        ot = opool.tile([128, TQ], F32, tag="ot",
                                        name=f"ot{t}_{nb}")
                        nc.vector.tensor_copy(ot[:], ps3[:])
                        outq[(t + nb) % 2].dma_start(
                            out_d[t * 128:(t + 1) * 128, nsl], ot[:])
                    items.append((None, fn, 4 * EST_FILL))
            return items

        outq = [nc.gpsimd, nc.scalar]
        norm_q = deque()   # deferred [broadcast + yT mul] closures

        def drain_norm():
            while norm_q:
                norm_q.popleft()()
                est["pe"] += 2 * EST_FILL

        # ---------------- main pipelined loop ----------------
        xload(0)
        if causal:
            xload(1)
            filler.extend(ph1_steps(0))
        else:
            for j in range(1, NQB):
                xload(j)
            for j in range(NQB):
                filler.extend(ph1_steps(j))

        for j in range(NQB):
            jsl = slice(j * TQ, (j + 1) * TQ)
            cs = list(range(4 * (j + 1))) if causal else list(range(NKC))
            if causal and j + 2 < NQB:
                xload(j + 2)
            if causal and j + 1 < NQB:
                filler.extend(ph1_steps(j + 1))

            for i in range(4):          # head pair (2i, 2i+1)
                hA, hB = 2 * i, 2 * i + 1
                need(("m", j, i))
                poA = ps_o.tile([128, TQ], F32, tag="po", name=f"poA{j}_{i}")
                poB = ps_o.tile([128, TQ], F32, tag="po", name=f"poB{j}_{i}")

                pend = deque()  # pipeline: PV(c) emitted after QK(c+2)
                first_pv = [True]

                def pv_pop(stop):
                    pc, ppt = pend.popleft()
                    need(("v", pc // 4, pc % 4))
                    pv_emit(pc, ppt, first_pv[0], stop)
                    first_pv[0] = False
                def qskip(c):
                    # fully-masked leading query columns of a diagonal chunk
                    if causal and c >= 4 * j:
                        return (c - 4 * j) * TKC
                    return 0

                def pv_emit(pc, ppt, start, stop):
                    k0 = qskip(pc)
                    nc.tensor.matmul(
                        poA[:, k0:TQ], vs[pc][:, hA * 65:hA * 65 + 128],
                        ppt[:, k0:TQ], start=start, stop=stop)
                    nc.tensor.matmul(
                        poB[:, k0:TQ], vs[pc][:, hB * 65:hB * 65 + 128],
                        ppt[:, TQ + k0:2 * TQ], start=start, stop=stop)
                    est["pe"] += EST_PV

                for ci, c in enumerate(cs):
                    need(("m", c // 4, 4 + i))
                    csl = slice(c * TKC, (c + 1) * TKC)
                    k0 = qskip(c)
                    ss = ps_s.tile([TKC, 2 * TQ], F32, tag="ss",
                                   name=f"ss{j}_{i}_{c}")
                    nc.tensor.matmul(ss[:, k0:TQ], kT[i][0:64, csl],
                                     q2[i][0:64, j * TQ + k0:(j + 1) * TQ],
                                     start=True, stop=True)
                    nc.tensor.matmul(ss[:, TQ + k0:2 * TQ], kT[i][64:128, csl],
                                     q2[i][64:128, j * TQ + k0:(j + 1) * TQ],
                                     start=True, stop=True)
                    est["pe"] += EST_QK
                    pt = ppool.tile([TKC, 2 * TQ], BF16, tag="pt",
                                    name=f"pt{j}_{i}_{c}")
                    nc.scalar.activation(pt[:], ss[:], EXP, scale=0.125)
                    est["act"] += EST_EXP
                    if causal and c >= 4 * j:
                        # only the 128-wide diagonal band is partially masked
                        bsl = slice(k0, k0 + TKC)
                        nc.vector.tensor_mul(pt[:, bsl], pt[:, bsl], maskb[:])
                        bslB = slice(TQ + k0, TQ + k0 + TKC)
                        nc.vector.tensor_mul(pt[:, bslB], pt[:, bslB],
                                             maskb[:])
                    if ci == 2:
                        drain_norm()   # previous pair's deferred norm
                    if len(pend) >= 2:
                        pv_pop(False)
                    pend.append((c, pt))
                    budget_pops()
                if len(cs) < 3:
                    drain_norm()
                while len(pend) > 1:
                    pv_pop(False)
                budget_pops()
                pv_pop(True)

                # immediate DVE part of softmax normalization
                rr, osb = [], []
                for h, po in ((hA, poA), (hB, poB)):
                    sums = npool.tile([1, TQ], F32, tag="sums",
                                      name=f"sm{j}_{h}")
                    nc.vector.tensor_copy(sums[:], po[64:65, :])
                    o_sb = npool.tile([64, TQ], BF16, tag="o_sb",
                                      name=f"ob{j}_{h}")
                    nc.vector.tensor_copy(o_sb[:], po[0:64, :])
                    recip = npool.tile([1, TQ], F32, tag="recip",
                                       name=f"rc{j}_{h}")
                    recip_r = npool.tile([1, TQ], F32R, tag="recip_r",
                                         name=f"rr{j}_{h}")
                    nc.vector.reciprocal_approx_fast(
                        out=recip[:], in_=sums[:])
                    nc.vector.tensor_copy(recip_r[:], recip[:])
                    rr.append(recip_r)
                    osb.append(o_sb)

                def norm_fn(i=i, jsl=jsl, osb=osb, rr=rr, j=j, hA=hA):
                    for hp in (0, 1):
                        pb = ps_sh.tile([64, TQ], F32, tag="sh",
                                        name=f"pb{j}_{hA + hp}")
                        nc.tensor.matmul(pb[:], ones_r[:, 0:64], rr[hp][:],
                                         start=True, stop=True)
                        nc.vector.tensor_mul(
                            yT[i][hp * 64:(hp + 1) * 64, jsl],
                            osb[hp][:], pb[:])
                norm_q.append(norm_fn)

            drain_norm()               # before proj(j) can be emitted
            filler.extend(proj_steps(j))

        while filler:                  # flush remaining projection work
            pop_one()

    nc.compile()
    return nc


def _get_nc(causal: bool, qkv_bias: bool = False):
    key = (causal, qkv_bias)
    if key not in _CACHE:
        _CACHE[key] = _build(causal, qkv_bias)
    return _CACHE[key]


def _host_masks() -> np.ndarray:
    i = np.arange(TKC)[:, None]
    jj = np.arange(TKC)[None, :]
    return np.ascontiguousarray(
        (jj >= i).astype(np.float32).astype(ml_dtypes.bfloat16))


def _make_in_maps(x, W_qkv, b_qkv, W_proj):
    masks_np = _host_masks()
    in_maps = []
    for core in range(N_CORES):
        b, g = core // 2, core % 2
        qc = slice(g * DL, (g + 1) * DL)
        kc = slice(D + g * DL, D + (g + 1) * DL)
        vc = slice(2 * D + g * DL, 2 * D + (g + 1) * DL)
        bf = ml_dtypes.bfloat16
        in_maps.append({
            "xT": np.ascontiguousarray(x[b].T.astype(bf)),
            "wqk": np.ascontiguousarray(
                np.concatenate([W_qkv[:, qc], W_qkv[:, kc]],
                               axis=1).astype(bf)),
            "wv": np.ascontiguousarray(W_qkv[:, vc].astype(bf)),
            "bqk": np.ascontiguousarray(
                np.concatenate([b_qkv[qc], b_qkv[kc]]).reshape(1, 2 * DL)),
            "bv": np.ascontiguousarray(b_qkv[vc].reshape(1, DL)),
            "wproj": np.ascontiguousarray(
                W_proj[g * DL:(g + 1) * DL, :].astype(bf)),
            "masks": masks_np,
        })
    return in_maps


def kernel(x, mask, W_qkv, b_qkv, W_proj, b_proj):
    x = np.asarray(x, dtype=np.float32)
    mask2d = np.asarray(mask, dtype=np.int32).reshape(T, T)
    W_qkv = np.asarray(W_qkv, dtype=np.float32)
    b_qkv = np.asarray(b_qkv, dtype=np.float32)
    W_proj = np.asarray(W_proj, dtype=np.float32)
    b_proj = np.asarray(b_proj, dtype=np.float32)

    if np.array_equal(mask2d, np.tril(np.ones((T, T), dtype=np.int32))):
        causal = True
    elif np.all(mask2d == 1):
        causal = False
    else:
        raise NotImplementedError("only causal (tril) or all-ones masks")

    qkv_bias = bool(np.any(b_qkv != 0.0))
    nc = _get_nc(causal, qkv_bias)
    in_maps = _make_in_maps(x, W_qkv, b_qkv, W_proj)
    res = run_bass_kernel_spmd(nc, in_maps, core_ids=list(range(N_CORES)))
    out = np.empty((B, T, D), dtype=np.float32)
    for b in range(B):
        out[b] = (res.results[2 * b]["out"] + res.results[2 * b + 1]["out"]
                  + b_proj[None, :])
    return out


# revision 13
# speedup vs baseline: 1.7907x; 1.2176x over previous
"""Multi-head causal self-attention for TRN2, 8 NeuronCores.

Sharding: core i handles (batch b = i//2, head-group g = i%2); each head-group
is 8 of the 16 heads.  Per core everything is computed in "transposed" space so
no on-device transposes are needed.

v2 vs baseline:
  * QK attention matmuls are row-tiled: the two heads of a pair run as
    concurrent K=64 matmuls on PE row-groups (0,0)/(64,0) writing the two
    halves (separate PSUM banks) of one [128, 1024] score tile, so both
    heads' S^T cost one 512-col stream instead of two.
  * Single software-pipelined loop: phase-1 QKV projection work for block
    j+1 and the output projection for block j-1 are emitted as fine-grained
    "filler" matmuls inside the ACT(exp)-paced attention chunk loop, with
    FIFO gating so attention never waits on un-emitted producers.
  * No ACT in phase 1: QKV biases (when nonzero) are rank-1 matmuls; PSUM
    drains are DVE copies.  ACT does only the 160 softmax exp calls.
  * x^T f32->bf16 casts run on the otherwise-idle GPSIMD engine.
  * softmax normalization: reciprocal on DVE, [1,q]->[64,q] broadcast via a
    K=1 matmul into the shared PSUM pool, deferred past the next pair's
    first chunk so the PE never stalls on the DVE reciprocal chain.
"""

import numpy as np
import ml_dtypes
from collections import deque
from contextlib import ExitStack

import concourse.bass as bass
import concourse.mybir as mybir
import concourse.tile as tile
from concourse import bacc
from concourse.bass_utils import run_bass_kernel_spmd

B, T, D, H = 4, 2048, 1024, 16
DK = 64            # head dim
HL = 8             # heads per core
DL = HL * DK       # 512 local head dims per core
N_CORES = 8

F32 = mybir.dt.float32
F32R = mybir.dt.float32r
BF16 = mybir.dt.bfloat16
EXP = mybir.ActivationFunctionType.Exp

TQ = 512           # tq block size
TKC = 128          # tk chunk size
NQB = T // TQ      # 4
NKC = T // TKC     # 16
NDCH = D // 128    # 8 contraction chunks over D
VSW = HL * 65 + 64  # staged-V width: 8*[V_h|1] + ones tail pad for M=128 lhsT

# rough per-instruction engine-busy estimates (ns) for the static scheduler
EST_QK = 320       # two concurrent row-tiled K=64 matmuls, N=512
EST_PV = 450       # two K=128 matmuls, N=512
EST_EXP = 1005     # ACT exp on [128, 1024] (measured)
EST_FILL = 230     # one N=512 matmul
RESERVE = 400

_CACHE = {}


def _build(causal: bool, qkv_bias: bool):
    nc = bacc.Bacc("TRN2", target_bir_lowering=False, debug=False,
                   num_devices=N_CORES)
    xT_d = nc.dram_tensor("xT", [D, T], BF16, kind="ExternalInput").ap()
    wqk_d = nc.dram_tensor("wqk", [D, 2 * DL], BF16, kind="ExternalInput").ap()
    wv_d = nc.dram_tensor("wv", [D, DL], BF16, kind="ExternalInput").ap()
    wp_d = nc.dram_tensor("wproj", [DL, D], BF16, kind="ExternalInput").ap()
    bqk_d = nc.dram_tensor("bqk", [1, 2 * DL], F32, kind="ExternalInput").ap()
    bv_d = nc.dram_tensor("bv", [1, DL], F32, kind="ExternalInput").ap()
    masks_d = nc.dram_tensor("masks", [TKC, TKC], BF16,
                             kind="ExternalInput").ap()
    out_d = nc.dram_tensor("out", [T, D], F32, kind="ExternalOutput").ap()

    with tile.TileContext(nc) as tc, ExitStack() as top:
        persist = top.enter_context(tc.tile_pool(name="persist", bufs=1))
        xrpool = top.enter_context(
            tc.tile_pool(name="xrpool", bufs=2 if causal else 4))
        ps_s = top.enter_context(tc.tile_pool(name="ps_s", bufs=2, space="PSUM"))
        ps_o = top.enter_context(tc.tile_pool(name="ps_o", bufs=2, space="PSUM"))
        ps_sh = top.enter_context(tc.tile_pool(name="ps_sh", bufs=2, space="PSUM"))
        ppool = top.enter_context(tc.tile_pool(name="ppool", bufs=8))
        npool = top.enter_context(tc.tile_pool(name="npool", bufs=4))
        opool = top.enter_context(tc.tile_pool(name="opool", bufs=3))

        # ---------------- persistent tiles ----------------
        q2 = [persist.tile([128, T], BF16, tag=f"q2{i}", name=f"q2{i}")
              for i in range(4)]       # head-pair packed Q^T
        kT = [persist.tile([128, T], BF16, tag=f"kT{i}", name=f"kT{i}")
              for i in range(4)]       # head-pair packed K^T
        vs = [persist.tile([128, VSW], BF16, tag=f"vs{t}", name=f"vs{t}")
              for t in range(NKC)]     # staged V: [V_h|1]*8 + ones tail
        yT = [persist.tile([128, T], BF16, tag=f"yT{i}", name=f"yT{i}")
              for i in range(4)]
        wqk_r = [persist.tile([128, 2 * DL], BF16, tag=f"wqk{d}", name=f"wqk{d}")
                 for d in range(NDCH)]
        wv_r = [persist.tile([128, DL], BF16, tag=f"wv{d}", name=f"wv{d}")
                for d in range(NDCH)]
        wp_r = [persist.tile([128, D], BF16, tag=f"wp{k}", name=f"wp{k}")
                for k in range(4)]
        ones_r = persist.tile([1, 128], F32R, tag="ones_r", name="ones_r")
        maskb = None
        if causal:
            maskb = persist.tile([TKC, TKC], BF16, tag="maskb", name="maskb")
            nc.gpsimd.dma_start(maskb[:], masks_d)

        # ---------------- preamble ----------------
        initp = top.enter_context(tc.tile_pool(name="initp", bufs=1))
        ones_f = initp.tile([1, 512], F32, tag="ones_f", name="ones_f")
        nc.vector.memset(ones_f[:], 1.0)
        nc.vector.tensor_copy(ones_r[:], ones_f[:, 0:128])
        bqk_r = bv_r = ones512_r = None
        if qkv_bias:
            ones512_r = initp.tile([1, 512], F32R, tag="ones512_r",
                                   name="ones512_r")
            nc.vector.tensor_copy(ones512_r[:], ones_f[:])
            bqk_f = initp.tile([1, 2 * DL], F32, tag="bqk_f", name="bqk_f")
            nc.gpsimd.dma_start(bqk_f[:], bqk_d)
            bqk_r = initp.tile([1, 2 * DL], F32R, tag="bqk_r", name="bqk_r")
            nc.vector.tensor_copy(bqk_r[:], bqk_f[:])
            bv_f = initp.tile([1, DL], F32, tag="bv_f", name="bv_f")
            nc.gpsimd.dma_start(bv_f[:], bv_d)
            bv_r = initp.tile([1, DL], F32R, tag="bv_r", name="bv_r")
            nc.vector.tensor_copy(bv_r[:], bv_f[:])

        # weights: DMA stage f32 -> DVE cast to bf16 resident copies
        dmaq = [nc.gpsimd, nc.scalar]
        for d in range(NDCH):
            dmaq[d % 2].dma_start(wqk_r[d][:], wqk_d[d * 128:(d + 1) * 128, :])
        for d in range(NDCH):
            dmaq[d % 2].dma_start(wv_r[d][:], wv_d[d * 128:(d + 1) * 128, :])
        for k in range(4):
            dmaq[k % 2].dma_start(wp_r[k][:], wp_d[k * 128:(k + 1) * 128, :])

        # staged-V tiles start as all-ones; the V copies overwrite the V
        # columns and leave the |1 columns and the tail as ones.
        for t in range(NKC):
            nc.vector.memset(vs[t][:], 1.0)

        # ---------------- x loads (DMA + gpsimd cast) ----------------
        xr_cache = {}

        def xload(j):
            jsl = slice(j * TQ, (j + 1) * TQ)
            xr_j = []
            for d in range(NDCH):
                xr_t = xrpool.tile([128, TQ], BF16, tag=f"xr{d}",
                                   name=f"xr{j}_{d}")
                nc.sync.dma_start(xr_t[:], xT_d[d * 128:(d + 1) * 128, jsl])
                xr_j.append(xr_t)
            xr_cache[j] = xr_j

        # ---------------- filler machinery ----------------
        filler = deque()   # items: (label_or_None, fn, est_pe_ns)
        done = set()
        est = {"pe": 0.0, "act": 0.0}

        def pop_one():
            label, fn, cost = filler.popleft()
            fn()
            if label is not None:
                done.add(label)
            est["pe"] += cost

        def need(label):
            while label not in done:
                assert filler, f"gate {label} not in filler"
                pop_one()

        def budget_pops():
            while filler and est["pe"] + RESERVE < est["act"]:
                pop_one()

        def ph1_steps(j):
            """Phase-1 QKV projection for query block j as filler items."""
            jsl = slice(j * TQ, (j + 1) * TQ)
            xr_j = xr_cache[j]
            items = []

            def m_group(m):
                cell = {}

                def mk(d):
                    def fn():
                        if d == 0:
                            cell["ps"] = ps_sh.tile(
                                [128, TQ], F32, tag="sh", name=f"psqk{j}_{m}")
                        ps = cell["ps"]
                        last = (d == NDCH - 1) and not qkv_bias
                        nc.tensor.matmul(
                            ps[:], wqk_r[d][:, m * 128:(m + 1) * 128],
                            xr_j[d][:], start=(d == 0), stop=last)
                        if d == NDCH - 1:
                            if qkv_bias:
                                nc.tensor.matmul(
                                    ps[:], bqk_r[0:1, m * 128:(m + 1) * 128],
                                    ones512_r[:], start=False, stop=True)
                            dst = q2[m] if m < 4 else kT[m - 4]
                            nc.vector.tensor_copy(dst[:, jsl], ps[:])
                    return fn

                return ([(None, mk(d), EST_FILL) for d in range(NDCH - 1)]
                        + [(("m", j, m), mk(NDCH - 1), EST_FILL + 60)])

            def v_group(tt):
                c = tt % 4
                cell = {}

                def mk(d):
                    def fn():
                        if d == 0:
                            cell["ps"] = ps_sh.tile(
                                [128, DL], F32, tag="sh", name=f"psv{tt}")
                        ps = cell["ps"]
                        last = (d == NDCH - 1) and not qkv_bias
                        nc.tensor.matmul(
                            ps[:], xr_j[d][:, c * 128:(c + 1) * 128],
                            wv_r[d][:], start=(d == 0), stop=last)
                        if d == NDCH - 1:
                            if qkv_bias:
                                nc.tensor.matmul(
                                    ps[:], ones_r[:, 0:128], bv_r[:],
                                    start=False, stop=True)
                            src = ps.rearrange("p (h x) -> p h x", h=HL)
                            dst = vs[tt][:, 0:HL * 65].rearrange(
                                "p (h x) -> p h x", x=65)[:, :, 0:64]
                            nc.vector.tensor_copy(dst, src)
                    return fn

                return ([(None, mk(d), EST_FILL) for d in range(NDCH - 1)]
                        + [(("v", j, tt % 4), mk(NDCH - 1), EST_FILL + 60)])

            # order: what attention needs first -- pair-0 Q/K, then V, then
            # the remaining pairs' Q/K.
            items += m_group(0) + m_group(4)
            for tt in range(4 * j, 4 * j + 4):
                items += v_group(tt)
            for i in range(1, 4):
                items += m_group(i) + m_group(4 + i)
            return items

        def proj_steps(j):
            items = []
            for t in range(4 * j, 4 * j + 4):
                for nb in range(2):
                    def fn(t=t, nb=nb):
                        nsl = slice(nb * 512, (nb + 1) * 512)
                        ps3 = ps_sh.tile([128, TQ], F32, tag="sh",
                                         name=f"ps3_{t}_{nb}")
                        for k in range(4):
                            nc.tensor.matmul(
                                ps3[:], yT[k][:, t * 128:(t + 1) * 128],
                                wp_r[k][:, nsl], start=(k == 0), stop=(k == 3))
                        ot = opool.tile([128, TQ], F32, tag="ot",
                                        name=f"ot{t}_{nb}")
                        nc.vector.tensor_copy(ot[:], ps3[:])
                        outq[(t + nb) % 2].dma_start(
                            out_d[t * 128:(t + 1) * 128, nsl], ot[:])
                    items.append((None, fn, 4 * EST_FILL))
            return items

        outq = [nc.gpsimd, nc.scalar]
        pair_no = [0]      # global head-pair counter (norm gating)

        # ---------------- main pipelined loop ----------------
        xload(0)
        if causal:
            xload(1)
            filler.extend(ph1_steps(0))
        else:
            for j in range(1, NQB):
                xload(j)
            for j in range(NQB):
                filler.extend(ph1_steps(j))

        for j in range(NQB):
            jsl = slice(j * TQ, (j + 1) * TQ)
            cs = list(range(4 * (j + 1))) if causal else list(range(NKC))
            if causal and j + 2 < NQB:
                xload(j + 2)
            if causal and j + 1 < NQB:
                filler.extend(ph1_steps(j + 1))

            for i in range(4):          # head pair (2i, 2i+1)
                hA, hB = 2 * i, 2 * i + 1
                need(("m", j, i))
                poA = ps_o.tile([128, TQ], F32, tag="po", name=f"poA{j}_{i}")
                poB = ps_o.tile([128, TQ], F32, tag="po", name=f"poB{j}_{i}")

                pend = deque()  # pipeline: PV(c) emitted after QK(c+2)
                first_pv = [True]

                def pv_pop(stop):
                    pc, ppt = pend.popleft()
                    need(("v", pc // 4, pc % 4))
                    pv_emit(pc, ppt, first_pv[0], stop)
                    first_pv[0] = False
                def qskip(c):
                    # fully-masked leading query columns of a diagonal chunk
                    if causal and c >= 4 * j:
                        return (c - 4 * j) * TKC
                    return 0

                def pv_emit(pc, ppt, start, stop):
                    k0 = qskip(pc)
                    nc.tensor.matmul(
                        poA[:, k0:TQ], vs[pc][:, hA * 65:hA * 65 + 128],
                        ppt[:, k0:TQ], start=start, stop=stop)
                    nc.tensor.matmul(
                        poB[:, k0:TQ], vs[pc][:, hB * 65:hB * 65 + 128],
                        ppt[:, TQ + k0:2 * TQ], start=start, stop=stop)
                    est["pe"] += EST_PV

                for ci, c in enumerate(cs):
                    need(("m", c // 4, 4 + i))
                    csl = slice(c * TKC, (c + 1) * TKC)
                    k0 = qskip(c)
                    ss = ps_s.tile([TKC, 2 * TQ], F32, tag="ss",
                                   name=f"ss{j}_{i}_{c}")
                    nc.tensor.matmul(ss[:, k0:TQ], kT[i][0:64, csl],
                                     q2[i][0:64, j * TQ + k0:(j + 1) * TQ],
                                     start=True, stop=True)
                    nc.tensor.matmul(ss[:, TQ + k0:2 * TQ], kT[i][64:128, csl],
                                     q2[i][64:128, j * TQ + k0:(j + 1) * TQ],
                                     start=True, stop=True)
                    est["pe"] += EST_QK
                    pt = ppool.tile([TKC, 2 * TQ], BF16, tag="pt",
                                    name=f"pt{j}_{i}_{c}")
                    nc.scalar.activation(pt[:], ss[:], EXP, scale=0.125)
                    est["act"] += EST_EXP
                    if causal and c >= 4 * j:
                        # only the 128-wide diagonal band is partially masked
                        bsl = slice(k0, k0 + TKC)
                        nc.vector.tensor_mul(pt[:, bsl], pt[:, bsl], maskb[:])
                        bslB = slice(TQ + k0, TQ + k0 + TKC)
                        nc.vector.tensor_mul(pt[:, bslB], pt[:, bslB],
                                             maskb[:])
                    if len(pend) >= 2:
                        pv_pop(False)
                    pend.append((c, pt))
                    budget_pops()
                while len(pend) > 1:
                    pv_pop(False)
                budget_pops()
                pv_pop(True)

                # immediate DVE part of softmax normalization; the
                # broadcast+multiply is queued as a filler item so the PE
                # stream never pauses at pair/phase boundaries.
                if pair_no[0] >= 2:
                    need(("n", pair_no[0] - 2))   # npool buffer rotation
                rr, osb = [], []
                for h, po in ((hA, poA), (hB, poB)):
                    o_sb = npool.tile([64, TQ], BF16, tag="o_sb",
                                      name=f"ob{j}_{h}")
                    nc.vector.tensor_copy(o_sb[:], po[0:64, :])
                    sums = npool.tile([1, TQ], F32, tag="sums",
                                      name=f"sm{j}_{h}")
                    nc.vector.tensor_copy(sums[:], po[64:65, :])
                    recip = npool.tile([1, TQ], F32, tag="recip",
                                       name=f"rc{j}_{h}")
                    nc.vector.reciprocal_approx_fast(
                        out=recip[:], in_=sums[:])
                    rr.append(recip)
                    osb.append(o_sb)

                def norm_fn(i=i, jsl=jsl, osb=osb, rr=rr, j=j, hA=hA):
                    for hp in (0, 1):
                        pb = npool.tile([64, TQ], F32, tag="pb",
                                        name=f"pb{j}_{hA + hp}")
                        nc.gpsimd.partition_broadcast(pb[:], rr[hp][:])
                        nc.vector.tensor_mul(
                            yT[i][hp * 64:(hp + 1) * 64, jsl],
                            osb[hp][:], pb[:])
                filler.append((("n", pair_no[0]), norm_fn, 50))
                pair_no[0] += 1

            filler.extend(proj_steps(j))

        while filler:                  # flush remaining projection work
            pop_one()

    nc.compile()
    return nc


def _get_nc(causal: bool, qkv_bias: bool = False):
    key = (causal, qkv_bias)
    if key not in _CACHE:
        _CACHE[key] = _build(causal, qkv_bias)
    return _CACHE[key]


def _host_masks() -> np.ndarray:
    i = np.arange(TKC)[:, None]
    jj = np.arange(TKC)[None, :]
    return np.ascontiguousarray(
        (jj >= i).astype(np.float32).astype(ml_dtypes.bfloat16))


def _make_in_maps(x, W_qkv, b_qkv, W_proj):
    masks_np = _host_masks()
    in_maps = []
    for core in range(N_CORES):
        b, g = core // 2, core % 2
        qc = slice(g * DL, (g + 1) * DL)
        kc = slice(D + g * DL, D + (g + 1) * DL)
        vc = slice(2 * D + g * DL, 2 * D + (g + 1) * DL)
        bf = ml_dtypes.bfloat16
        in_maps.append({
            "xT": np.ascontiguousarray(x[b].T.astype(bf)),
            "wqk": np.ascontiguousarray(
                np.concatenate([W_qkv[:, qc], W_qkv[:, kc]],
                               axis=1).astype(bf)),
            "wv": np.ascontiguousarray(W_qkv[:, vc].astype(bf)),
            "bqk": np.ascontiguousarray(
                np.concatenate([b_qkv[qc], b_qkv[kc]]).reshape(1, 2 * DL)),
            "bv": np.ascontiguousarray(b_qkv[vc].reshape(1, DL)),
            "wproj": np.ascontiguousarray(
                W_proj[g * DL:(g + 1) * DL, :].astype(bf)),
            "masks": masks_np,
        })
    return in_maps


def kernel(x, mask, W_qkv, b_qkv, W_proj, b_proj):
    x = np.asarray(x, dtype=np.float32)
    mask2d = np.asarray(mask, dtype=np.int32).reshape(T, T)
    W_qkv = np.asarray(W_qkv, dtype=np.float32)
    b_qkv = np.asarray(b_qkv, dtype=np.float32)
    W_proj = np.asarray(W_proj, dtype=np.float32)
    b_proj = np.asarray(b_proj, dtype=np.float32)

    if np.array_equal(mask2d, np.tril(np.ones((T, T), dtype=np.int32))):
        causal = True
    elif np.all(mask2d == 1):
        causal = False
    else:
        raise NotImplementedError("only causal (tril) or all-ones masks")

    qkv_bias = bool(np.any(b_qkv != 0.0))
    nc = _get_nc(causal, qkv_bias)
    in_maps = _make_in_maps(x, W_qkv, b_qkv, W_proj)
    res = run_bass_kernel_spmd(nc, in_maps, core_ids=list(range(N_CORES)))
    out = np.empty((B, T, D), dtype=np.float32)
    for b in range(B):
        out[b] = (res.results[2 * b]["out"] + res.results[2 * b + 1]["out"]
                  + b_proj[None, :])
    return out


# revision 14
# speedup vs baseline: 1.8130x; 1.0125x over previous
"""Multi-head causal self-attention for TRN2, 8 NeuronCores.

Sharding: core i handles (batch b = i//2, head-group g = i%2); each head-group
is 8 of the 16 heads.  Per core everything is computed in "transposed" space so
no on-device transposes are needed.

v2 vs baseline:
  * QK attention matmuls are row-tiled: the two heads of a pair run as
    concurrent K=64 matmuls on PE row-groups (0,0)/(64,0) writing the two
    halves (separate PSUM banks) of one [128, 1024] score tile, so both
    heads' S^T cost one 512-col stream instead of two.
  * Single software-pipelined loop: phase-1 QKV projection work for block
    j+1 and the output projection for block j-1 are emitted as fine-grained
    "filler" matmuls inside the ACT(exp)-paced attention chunk loop, with
    FIFO gating so attention never waits on un-emitted producers.
  * No ACT in phase 1: QKV biases (when nonzero) are rank-1 matmuls; PSUM
    drains are DVE copies.  ACT does only the 160 softmax exp calls.
  * x^T f32->bf16 casts run on the otherwise-idle GPSIMD engine.
  * softmax normalization: reciprocal on DVE, [1,q]->[64,q] broadcast via a
    K=1 matmul into the shared PSUM pool, deferred past the next pair's
    first chunk so the PE never stalls on the DVE reciprocal chain.
"""

import numpy as np
import ml_dtypes
from collections import deque
from contextlib import ExitStack

import concourse.bass as bass
import concourse.mybir as mybir
import concourse.tile as tile
from concourse import bacc
from concourse.bass_utils import run_bass_kernel_spmd

B, T, D, H = 4, 2048, 1024, 16
DK = 64            # head dim
HL = 8             # heads per core
DL = HL * DK       # 512 local head dims per core
N_CORES = 8

F32 = mybir.dt.float32
F32R = mybir.dt.float32r
BF16 = mybir.dt.bfloat16
EXP = mybir.ActivationFunctionType.Exp

TQ = 512           # tq block size
TKC = 128          # tk chunk size
NQB = T // TQ      # 4
NKC = T // TKC     # 16
NDCH = D // 128    # 8 contraction chunks over D
VSW = HL * 65 + 64  # staged-V width: 8*[V_h|1] + ones tail pad for M=128 lhsT

# rough per-instruction engine-busy estimates (ns) for the static scheduler
EST_QK = 320       # two concurrent row-tiled K=64 matmuls, N=512
EST_PV = 450       # two K=128 matmuls, N=512
EST_EXP = 1050     # ACT exp on [128, 1024] (measured)
EST_FILL = 230     # one N=512 matmul
RESERVE = 300

_CACHE = {}


def _build(causal: bool, qkv_bias: bool):
    nc = bacc.Bacc("TRN2", target_bir_lowering=False, debug=False,
                   num_devices=N_CORES)
    xT_d = nc.dram_tensor("xT", [D, T], BF16, kind="ExternalInput").ap()
    wqk_d = nc.dram_tensor("wqk", [D, 2 * DL], BF16, kind="ExternalInput").ap()
    wv_d = nc.dram_tensor("wv", [D, DL], BF16, kind="ExternalInput").ap()
    wp_d = nc.dram_tensor("wproj", [DL, D], BF16, kind="ExternalInput").ap()
    bqk_d = nc.dram_tensor("bqk", [1, 2 * DL], F32, kind="ExternalInput").ap()
    bv_d = nc.dram_tensor("bv", [1, DL], F32, kind="ExternalInput").ap()
    masks_d = nc.dram_tensor("masks", [TKC, TKC], BF16,
                             kind="ExternalInput").ap()
    out_d = nc.dram_tensor("out", [T, D], BF16, kind="ExternalOutput").ap()

    with tile.TileContext(nc) as tc, ExitStack() as top:
        persist = top.enter_context(tc.tile_pool(name="persist", bufs=1))
        xrpool = top.enter_context(
            tc.tile_pool(name="xrpool", bufs=2 if causal else 4))
        ps_s = top.enter_context(tc.tile_pool(name="ps_s", bufs=2, space="PSUM"))
        ps_o = top.enter_context(tc.tile_pool(name="ps_o", bufs=2, space="PSUM"))
        ps_sh = top.enter_context(tc.tile_pool(name="ps_sh", bufs=2, space="PSUM"))
        ppool = top.enter_context(tc.tile_pool(name="ppool", bufs=8))
        npool = top.enter_context(tc.tile_pool(name="npool", bufs=4))
        opool = top.enter_context(tc.tile_pool(name="opool", bufs=3))

        # ---------------- persistent tiles ----------------
        q2 = [persist.tile([128, T], BF16, tag=f"q2{i}", name=f"q2{i}")
              for i in range(4)]       # head-pair packed Q^T
        kT = [persist.tile([128, T], BF16, tag=f"kT{i}", name=f"kT{i}")
              for i in range(4)]       # head-pair packed K^T
        vs = [persist.tile([128, VSW], BF16, tag=f"vs{t}", name=f"vs{t}")
              for t in range(NKC)]     # staged V: [V_h|1]*8 + ones tail
        yT = [persist.tile([128, T], BF16, tag=f"yT{i}", name=f"yT{i}")
              for i in range(4)]
        wqk_r = [persist.tile([128, 2 * DL], BF16, tag=f"wqk{d}", name=f"wqk{d}")
                 for d in range(NDCH)]
        wv_r = [persist.tile([128, DL], BF16, tag=f"wv{d}", name=f"wv{d}")
                for d in range(NDCH)]
        wp_r = [persist.tile([128, D], BF16, tag=f"wp{k}", name=f"wp{k}")
                for k in range(4)]
        ones_r = persist.tile([1, 128], F32R, tag="ones_r", name="ones_r")
        maskb = None
        if causal:
            maskb = persist.tile([TKC, TKC], BF16, tag="maskb", name="maskb")
            nc.gpsimd.dma_start(maskb[:], masks_d)

        # ---------------- preamble ----------------
        initp = top.enter_context(tc.tile_pool(name="initp", bufs=1))
        ones_f = initp.tile([1, 512], F32, tag="ones_f", name="ones_f")
        nc.vector.memset(ones_f[:], 1.0)
        nc.vector.tensor_copy(ones_r[:], ones_f[:, 0:128])
        bqk_r = bv_r = ones512_r = None
        if qkv_bias:
            ones512_r = initp.tile([1, 512], F32R, tag="ones512_r",
                                   name="ones512_r")
            nc.vector.tensor_copy(ones512_r[:], ones_f[:])
            bqk_f = initp.tile([1, 2 * DL], F32, tag="bqk_f", name="bqk_f")
            nc.gpsimd.dma_start(bqk_f[:], bqk_d)
            bqk_r = initp.tile([1, 2 * DL], F32R, tag="bqk_r", name="bqk_r")
            nc.vector.tensor_copy(bqk_r[:], bqk_f[:])
            bv_f = initp.tile([1, DL], F32, tag="bv_f", name="bv_f")
            nc.gpsimd.dma_start(bv_f[:], bv_d)
            bv_r = initp.tile([1, DL], F32R, tag="bv_r", name="bv_r")
            nc.vector.tensor_copy(bv_r[:], bv_f[:])

        # weights: DMA stage f32 -> DVE cast to bf16 resident copies
        dmaq = [nc.gpsimd, nc.scalar, nc.sync]
        for d in range(NDCH):
            dmaq[d % 3].dma_start(wqk_r[d][:], wqk_d[d * 128:(d + 1) * 128, :])
        for d in range(NDCH):
            dmaq[d % 2].dma_start(wv_r[d][:], wv_d[d * 128:(d + 1) * 128, :])
        for k in range(4):
            dmaq[k % 2].dma_start(wp_r[k][:], wp_d[k * 128:(k + 1) * 128, :])

        # staged-V tiles start as all-ones; the V copies overwrite the V
        # columns and leave the |1 columns and the tail as ones.
        for t in range(NKC):
            nc.vector.memset(vs[t][:], 1.0)

        # ---------------- x loads (DMA + gpsimd cast) ----------------
        xr_cache = {}

        def xload(j):
            jsl = slice(j * TQ, (j + 1) * TQ)
            xr_j = []
            for d in range(NDCH):
                xr_t = xrpool.tile([128, TQ], BF16, tag=f"xr{d}",
                                   name=f"xr{j}_{d}")
                nc.sync.dma_start(xr_t[:], xT_d[d * 128:(d + 1) * 128, jsl])
                xr_j.append(xr_t)
            xr_cache[j] = xr_j

        # ---------------- filler machinery ----------------
        filler = deque()   # items: (label_or_None, fn, est_pe_ns)
        done = set()
        est = {"pe": 0.0, "act": 0.0}

        def pop_one():
            label, fn, cost = filler.popleft()
            fn()
            if label is not None:
                done.add(label)
            est["pe"] += cost

        def need(label):
            while label not in done:
                assert filler, f"gate {label} not in filler"
                pop_one()

        def budget_pops():
            while filler and est["pe"] + RESERVE < est["act"]:
                pop_one()

        def ph1_steps(j):
            """Phase-1 QKV projection for query block j as filler items."""
            jsl = slice(j * TQ, (j + 1) * TQ)
            xr_j = xr_cache[j]
            items = []

            def m_group(m):
                cell = {}

                def mk(d):
                    def fn():
                        if d == 0:
                            cell["ps"] = ps_sh.tile(
                                [128, TQ], F32, tag="sh", name=f"psqk{j}_{m}")
                        ps = cell["ps"]
                        last = (d == NDCH - 1) and not qkv_bias
                        nc.tensor.matmul(
                            ps[:], wqk_r[d][:, m * 128:(m + 1) * 128],
                            xr_j[d][:], start=(d == 0), stop=last)
                        if d == NDCH - 1:
                            if qkv_bias:
                                nc.tensor.matmul(
                                    ps[:], bqk_r[0:1, m * 128:(m + 1) * 128],
                                    ones512_r[:], start=False, stop=True)
                            dst = q2[m] if m < 4 else kT[m - 4]
                            nc.vector.tensor_copy(dst[:, jsl], ps[:])
                    return fn

                return ([(None, mk(d), EST_FILL) for d in range(NDCH - 1)]
                        + [(("m", j, m), mk(NDCH - 1), EST_FILL + 60)])

            def v_group(tt):
                c = tt % 4
                cell = {}

                def mk(d):
                    def fn():
                        if d == 0:
                            cell["ps"] = ps_sh.tile(
                                [128, DL], F32, tag="sh", name=f"psv{tt}")
                        ps = cell["ps"]
                        last = (d == NDCH - 1) and not qkv_bias
                        nc.tensor.matmul(
                            ps[:], xr_j[d][:, c * 128:(c + 1) * 128],
                            wv_r[d][:], start=(d == 0), stop=last)
                        if d == NDCH - 1:
                            if qkv_bias:
                                nc.tensor.matmul(
                                    ps[:], ones_r[:, 0:128], bv_r[:],
                                    start=False, stop=True)
                            src = ps.rearrange("p (h x) -> p h x", h=HL)
                            dst = vs[tt][:, 0:HL * 65].rearrange(
                                "p (h x) -> p h x", x=65)[:, :, 0:64]
                            nc.vector.tensor_copy(dst, src)
                    return fn

                return ([(None, mk(d), EST_FILL) for d in range(NDCH - 1)]
                        + [(("v", j, tt % 4), mk(NDCH - 1), EST_FILL + 60)])

            # order: what attention needs first -- pair-0 Q/K, then V, then
            # the remaining pairs' Q/K.
            items += m_group(0) + m_group(4)
            for tt in range(4 * j, 4 * j + 4):
                items += v_group(tt)
            for i in range(1, 4):
                items += m_group(i) + m_group(4 + i)
            return items

        def proj_steps(j):
            items = []
            for t in range(4 * j, 4 * j + 4):
                for nb in range(2):
                    def fn(t=t, nb=nb):
                        nsl = slice(nb * 512, (nb + 1) * 512)
                        ps3 = ps_sh.tile([128, TQ], F32, tag="sh",
                                         name=f"ps3_{t}_{nb}")
                        for k in range(4):
                            nc.tensor.matmul(
                                ps3[:], yT[k][:, t * 128:(t + 1) * 128],
                                wp_r[k][:, nsl], start=(k == 0), stop=(k == 3))
                        ot = opool.tile([128, TQ], BF16, tag="ot",
                                        name=f"ot{t}_{nb}")
                        nc.vector.tensor_copy(ot[:], ps3[:])
                        outq[(t + nb) % 2].dma_start(
                            out_d[t * 128:(t + 1) * 128, nsl], ot[:])
                    items.append((None, fn, 4 * EST_FILL))
            return items

        outq = [nc.gpsimd, nc.scalar]
        pair_no = [0]      # global head-pair counter (norm gating)

        # ---------------- main pipelined loop ----------------
        xload(0)
        if causal:
            xload(1)
            filler.extend(ph1_steps(0))
        else:
            for j in range(1, NQB):
                xload(j)
            for j in range(NQB):
                filler.extend(ph1_steps(j))

        for j in range(NQB):
            jsl = slice(j * TQ, (j + 1) * TQ)
            cs = list(range(4 * (j + 1))) if causal else list(range(NKC))
            if causal and j + 2 < NQB:
                xload(j + 2)
            if causal and j + 1 < NQB:
                filler.extend(ph1_steps(j + 1))

            for i in range(4):          # head pair (2i, 2i+1)
                hA, hB = 2 * i, 2 * i + 1
                need(("m", j, i))
                poA = ps_o.tile([128, TQ], F32, tag="po", name=f"poA{j}_{i}")
                poB = ps_o.tile([128, TQ], F32, tag="po", name=f"poB{j}_{i}")

                pend = deque()  # pipeline: PV(c) emitted after QK(c+2)
                first_pv = [True]

                def pv_pop(stop):
                    pc, ppt = pend.popleft()
                    need(("v", pc // 4, pc % 4))
                    pv_emit(pc, ppt, first_pv[0], stop)
                    first_pv[0] = False
                def qskip(c):
                    # fully-masked leading query columns of a diagonal chunk
                    if causal and c >= 4 * j:
                        return (c - 4 * j) * TKC
                    return 0

                def pv_emit(pc, ppt, start, stop):
                    k0 = qskip(pc)
                    nc.tensor.matmul(
                        poA[:, k0:TQ], vs[pc][:, hA * 65:hA * 65 + 128],
                        ppt[:, k0:TQ], start=start, stop=stop)
                    nc.tensor.matmul(
                        poB[:, k0:TQ], vs[pc][:, hB * 65:hB * 65 + 128],
                        ppt[:, TQ + k0:2 * TQ], start=start, stop=stop)
                    est["pe"] += EST_PV

                for ci, c in enumerate(cs):
                    need(("m", c // 4, 4 + i))
                    csl = slice(c * TKC, (c + 1) * TKC)
                    k0 = qskip(c)
                    ss = ps_s.tile([TKC, 2 * TQ], F32, tag="ss",
                                   name=f"ss{j}_{i}_{c}")
                    nc.tensor.matmul(ss[:, k0:TQ], kT[i][0:64, csl],
                                     q2[i][0:64, j * TQ + k0:(j + 1) * TQ],
                                     start=True, stop=True)
                    nc.tensor.matmul(ss[:, TQ + k0:2 * TQ], kT[i][64:128, csl],
                                     q2[i][64:128, j * TQ + k0:(j + 1) * TQ],
                                     start=True, stop=True)
                    est["pe"] += EST_QK
                    pt = ppool.tile([TKC, 2 * TQ], BF16, tag="pt",
                                    name=f"pt{j}_{i}_{c}")
                    nc.scalar.activation(pt[:], ss[:], EXP, scale=0.125)
                    est["act"] += EST_EXP
                    if causal and c >= 4 * j:
                        # only the 128-wide diagonal band is partially masked
                        bsl = slice(k0, k0 + TKC)
                        nc.vector.tensor_mul(pt[:, bsl], pt[:, bsl], maskb[:])
                        bslB = slice(TQ + k0, TQ + k0 + TKC)
                        nc.vector.tensor_mul(pt[:, bslB], pt[:, bslB],
                                             maskb[:])
                    if len(pend) >= 2:
                        pv_pop(False)
                    pend.append((c, pt))
                    budget_pops()
                while len(pend) > 1:
                    pv_pop(False)
                budget_pops()
                pv_pop(True)

                # immediate DVE part of softmax normalization; the
                # broadcast+multiply is queued as a filler item so the PE
                # stream never pauses at pair/phase boundaries.
                if pair_no[0] >= 2:
                    need(("n", pair_no[0] - 2))   # npool buffer rotation
                rr, osb = [], []
                for h, po in ((hA, poA), (hB, poB)):
                    o_sb = npool.tile([64, TQ], BF16, tag="o_sb",
                                      name=f"ob{j}_{h}")
                    nc.vector.tensor_copy(o_sb[:], po[0:64, :])
                    sums = npool.tile([1, TQ], F32, tag="sums",
                                      name=f"sm{j}_{h}")
                    nc.vector.tensor_copy(sums[:], po[64:65, :])
                    recip = npool.tile([1, TQ], F32, tag="recip",
                                       name=f"rc{j}_{h}")
                    nc.vector.reciprocal_approx_fast(
                        out=recip[:], in_=sums[:])
                    rr.append(recip)
                    osb.append(o_sb)

                def norm_fn(i=i, jsl=jsl, osb=osb, rr=rr, j=j, hA=hA):
                    for hp in (0, 1):
                        pb = npool.tile([64, TQ], F32, tag="pb",
                                        name=f"pb{j}_{hA + hp}")
                        nc.gpsimd.partition_broadcast(pb[:], rr[hp][:])
                        nc.vector.tensor_mul(
                            yT[i][hp * 64:(hp + 1) * 64, jsl],
                            osb[hp][:], pb[:])
                filler.append((("n", pair_no[0]), norm_fn, 50))
                pair_no[0] += 1

            filler.extend(proj_steps(j))

        while filler:                  # flush remaining projection work
            pop_one()

    nc.compile()
    return nc


def _get_nc(causal: bool, qkv_bias: bool = False):
    key = (causal, qkv_bias)
    if key not in _CACHE:
        _CACHE[key] = _build(causal, qkv_bias)
    return _CACHE[key]


def _host_masks() -> np.ndarray:
    i = np.arange(TKC)[:, None]
    jj = np.arange(TKC)[None, :]
    return np.ascontiguousarray(
        (jj >= i).astype(np.float32).astype(ml_dtypes.bfloat16))


def _make_in_maps(x, W_qkv, b_qkv, W_proj):
    masks_np = _host_masks()
    in_maps = []
    for core in range(N_CORES):
        b, g = core // 2, core % 2
        qc = slice(g * DL, (g + 1) * DL)
        kc = slice(D + g * DL, D + (g + 1) * DL)
        vc = slice(2 * D + g * DL, 2 * D + (g + 1) * DL)
        bf = ml_dtypes.bfloat16
        in_maps.append({
            "xT": np.ascontiguousarray(x[b].T.astype(bf)),
            "wqk": np.ascontiguousarray(
                np.concatenate([W_qkv[:, qc], W_qkv[:, kc]],
                               axis=1).astype(bf)),
            "wv": np.ascontiguousarray(W_qkv[:, vc].astype(bf)),
            "bqk": np.ascontiguousarray(
                np.concatenate([b_qkv[qc], b_qkv[kc]]).reshape(1, 2 * DL)),
            "bv": np.ascontiguousarray(b_qkv[vc].reshape(1, DL)),
            "wproj": np.ascontiguousarray(
                W_proj[g * DL:(g + 1) * DL, :].astype(bf)),
            "masks": masks_np,
        })
    return in_maps


def kernel(x, mask, W_qkv, b_qkv, W_proj, b_proj):
    x = np.asarray(x, dtype=np.float32)
    mask2d = np.asarray(mask, dtype=np.int32).reshape(T, T)
    W_qkv = np.asarray(W_qkv, dtype=np.float32)
    b_qkv = np.asarray(b_qkv, dtype=np.float32)
    W_proj = np.asarray(W_proj, dtype=np.float32)
    b_proj = np.asarray(b_proj, dtype=np.float32)

    if np.array_equal(mask2d, np.tril(np.ones((T, T), dtype=np.int32))):
        causal = True
    elif np.all(mask2d == 1):
        causal = False
    else:
        raise NotImplementedError("only causal (tril) or all-ones masks")

    qkv_bias = bool(np.any(b_qkv != 0.0))
    nc = _get_nc(causal, qkv_bias)
    in_maps = _make_in_maps(x, W_qkv, b_qkv, W_proj)
    res = run_bass_kernel_spmd(nc, in_maps, core_ids=list(range(N_CORES)))
    out = np.empty((B, T, D), dtype=np.float32)
    for b in range(B):
        out[b] = (res.results[2 * b]["out"].astype(np.float32)
                  + res.results[2 * b + 1]["out"].astype(np.float32)
                  + b_proj[None, :])
    return out
